# revision 5
# baseline (speedup 1.0000x reference)
"""Trainium2 Bass kernel for nn_EquivariantOutputHead.

Reference computation (B=8, T=32, R=512, D=256):
  x    = broadcast(scalar_features)                      (B,T,R,D)
  rel  = trans - mean_R(trans)
  lrp  = rotate(conj(normalize(quat)), rel)
  h1   = gelu([x, lrp] @ W1 + b1)
  h2   = gelu(h1 @ W2 + b2)
  tv   = rotate(normalize(quat), h2 @ Wt + bt)
  qv   = 0.5 * quat_mult(quat, (0, 0.1*(h2 @ Wr + br)))
  out  = [qv, tv]                                        (B,T,R,7)

Sharding: data-parallel over the 256 (b,t) pairs -> 32 pairs (16384 tokens)
per core.  sf @ W1[:D] + b1 is computed once per (b,t) (tiny matmul) and
folded into layer 1 as two bf16 hi/lo contraction rows (rhs rows = ones);
the layer-1 matmul is K=8 (3 lrp hi + 3 lrp lo + 2 c rows).

Rotations use v' = v +/- 2w(uxv)/|q|^2 + 2ux(uxv)/|q|^2 with raw quat
components - only 1/n2 (DVE reciprocal) is needed, no sqrt table.

The 32 groups are processed in two 16-group halves; all plane (DVE) work
for a half touches only its 64 partitions, so half-1 preprocessing and
half-0 output rotation overlap the matmul/gelu window of the other half.
"""

import os
import sys

for _p in ("/opt/trn_rl_repo",):
    if _p not in sys.path:
        sys.path.insert(0, _p)

import numpy as np

import concourse.bacc as bacc
import concourse.bass as bass
import concourse.mybir as mybir
import concourse.tile as tile
from concourse.bass_utils import run_bass_kernel_spmd

F32 = mybir.dt.float32
BF16 = mybir.dt.bfloat16
AF = mybir.ActivationFunctionType
OP = mybir.AluOpType
AX = mybir.AxisListType

B, T, R, D = 8, 32, 512, 256
NCORES = 8
PAIRS = B * T              # 256 (b,t) pairs
PPC = PAIRS // NCORES      # 32 pairs (groups) per core
TOK = PPC * R              # 16384 tokens per core
P = 128                    # partitions
NBLK = 8                   # uvT blocks (4 groups each)

GELU = AF.Gelu_apprx_tanh


def build_nc():
    nc = bacc.Bacc(None)

    quat_d = nc.declare_dram_parameter("quat", [P, 512], F32, isOutput=False)
    trans_d = nc.declare_dram_parameter("trans", [P, 384], F32, isOutput=False)
    sfTe_d = nc.declare_dram_parameter("sfTe", [257, PPC], F32, isOutput=False)
    w1aE_d = nc.declare_dram_parameter("W1aE", [257, 256], F32, isOutput=False)
    w1bc_d = nc.declare_dram_parameter("W1bc", [8, 8192], BF16, isOutput=False)
    ones_d = nc.declare_dram_parameter("ones2", [2, 16384], BF16, isOutput=False)
    w2_d = nc.declare_dram_parameter("W2", [256, 128], BF16, isOutput=False)
    b2t_d = nc.declare_dram_parameter("b2t", [P, 1], F32, isOutput=False)
    wtr_d = nc.declare_dram_parameter("Wtr", [P, 32], BF16, isOutput=False)
    btr_d = nc.declare_dram_parameter("btr", [P, 1], F32, isOutput=False)
    g_d = nc.declare_dram_parameter("G", [P, P], F32, isOutput=False)
    out_d = nc.declare_dram_parameter("out", [P, 896], F32, isOutput=True)

    with tile.TileContext(nc) as tc:
        with (
            tc.tile_pool(name="main", bufs=1) as main,
            tc.tile_pool(name="act", bufs=4) as actp,
            tc.tile_pool(name="h1p", bufs=18) as h1p,
            tc.tile_pool(name="ps1", bufs=2, space="PSUM") as ps1,
            tc.tile_pool(name="ps2", bufs=3, space="PSUM") as ps2,
            tc.tile_pool(name="ps3", bufs=1, space="PSUM") as ps3,
        ):
            # ---------- persistent SBUF tensors ----------
            qt = main.tile([P, 512], F32, tag="qt")     # raw quat, interleaved
            tt = main.tile([P, 384], F32, tag="tt")     # trans, interleaved
            sfTe = main.tile([P, 3 * PPC], F32, tag="sfTe")
            w1aE = main.tile([P, 768], F32, tag="w1aE")
            lhsT_all = main.tile([8, 8192], BF16, tag="lhsT_all")
            w2 = main.tile([P, 256], BF16, tag="w2")
            b2t = main.tile([P, 1], F32, tag="b2t")
            wtr = main.tile([P, 32], BF16, tag="wtr")
            btr = main.tile([P, 1], F32, tag="btr")
            g128 = main.tile([P, P], F32, tag="g128")

            cRhi = main.tile([PPC, 256], BF16, tag="cRhi")
            cRhf = main.tile([PPC, 256], F32, tag="cRhf")
            cRlo = main.tile([PPC, 256], BF16, tag="cRlo")
            S3 = main.tile([P, 3], F32, tag="S3")
            cent = main.tile([P, 3], F32, tag="cent")
            rel = main.tile([P, 384], F32, tag="rel")    # compact planes
            qc = main.tile([P, 512], F32, tag="qc")      # compact quat planes
            n2 = main.tile([P, P], F32, tag="n2")
            inv2 = main.tile([P, P], F32, tag="inv2")    # 1/|q|^2
            cr = main.tile([P, 384], F32, tag="cr")
            dd = main.tile([P, 384], F32, tag="dd")
            lrp = main.tile([P, 384], BF16, tag="lrp")
            tmpA = main.tile([P, P], F32, tag="tmpA")
            tmpB = main.tile([P, P], F32, tag="tmpB")
            tmpC = main.tile([P, P], F32, tag="tmpC")
            rhsT = main.tile([8, 16384], BF16, tag="rhsT")
            uvT = main.tile([P, 512 * NBLK], F32, tag="uvT")
            uvp = main.tile([P, 768], F32, tag="uvp")
            otile = main.tile([P, 896], F32, tag="otile")

            # ---------- loads (sync HWDGE; scalar stays pure-ACT) ----------
            nc.sync.dma_start(qt[:], quat_d[:])
            nc.sync.dma_start(tt[:], trans_d[:])
            nc.sync.dma_start(sfTe[:, 0:PPC], sfTe_d[0:128, :])
            nc.sync.dma_start(sfTe[:, PPC : 2 * PPC], sfTe_d[128:256, :])
            nc.sync.dma_start(sfTe[0:1, 2 * PPC : 3 * PPC], sfTe_d[256:257, :])
            nc.sync.dma_start(w1aE[:, 0:256], w1aE_d[0:128, :])
            nc.sync.dma_start(w1aE[:, 256:512], w1aE_d[128:256, :])
            nc.sync.dma_start(w1aE[0:1, 512:768], w1aE_d[256:257, :])
            nc.sync.dma_start(lhsT_all[:], w1bc_d[:])
            nc.sync.dma_start(rhsT[6:8, :], ones_d[:])
            nc.sync.dma_start(w2[:, 0:128], w2_d[0:128, :])
            nc.sync.dma_start(w2[:, 128:256], w2_d[128:256, :])
            nc.sync.dma_start(b2t[:], b2t_d[:])
            nc.sync.dma_start(wtr[:], wtr_d[:])
            nc.sync.dma_start(btr[:], btr_d[:])
            nc.sync.dma_start(g128[:], g_d[:])

            # ---------- cR = sf @ W1a + b1, token-major [32, 256] ----------
            psc2 = ps2.tile([PPC, 256], F32, tag="p2", name="psc2")
            nc.tensor.matmul(psc2[:], sfTe[:, 0:PPC], w1aE[:, 0:256],
                             start=True, stop=False)
            nc.tensor.matmul(psc2[:], sfTe[:, PPC : 2 * PPC], w1aE[:, 256:512],
                             start=False, stop=False)
            nc.tensor.matmul(psc2[:], sfTe[0:1, 2 * PPC : 3 * PPC],
                             w1aE[0:1, 512:768], start=False, stop=True)
            nc.vector.tensor_copy(cRhi[:], psc2[:])
            nc.vector.tensor_copy(cRhf[:], cRhi[:])
            nc.vector.tensor_sub(cRlo[:], psc2[:], cRhf[:])
            nc.sync.dma_start(lhsT_all[6:7, :], cRhi[:])
            nc.sync.dma_start(lhsT_all[7:8, :], cRlo[:])

            # ---------- centroid sums & quat prep (full planes, early) -----
            for c in range(3):
                nc.vector.reduce_sum(S3[:, c : c + 1], tt[:, c::3], axis=AX.X)
            for h in range(2):
                hp = slice(64 * h, 64 * h + 64)
                psch = ps2.tile([P, 3], F32, tag="p2", name="psc")
                nc.tensor.matmul(psch[hp, :], g128[hp, hp], S3[hp, :],
                                 start=True, stop=True,
                                 tile_position=(64 * h, 64 * h))
                nc.vector.tensor_scalar_mul(cent[hp, :], psch[hp, :], 1.0 / 512.0)
            for c in range(3):
                nc.vector.tensor_scalar_sub(
                    rel[:, P * c : P * (c + 1)], tt[:, c::3], cent[:, c : c + 1]
                )
            nc.vector.tensor_mul(n2[:], qt[:, 0::4], qt[:, 0::4])
            for c in range(1, 4):
                nc.vector.tensor_mul(tmpA[:], qt[:, c::4], qt[:, c::4])
                nc.vector.tensor_add(n2[:], n2[:], tmpA[:])
            nc.vector.reciprocal(inv2[:], n2[:])
            for c in range(4):
                nc.vector.tensor_copy(qc[:, P * c : P * (c + 1)], qt[:, c::4])

            def qp(c, hp):  # compact raw quat planes (0=w, 1..3=vec)
                return qc[hp, P * c : P * (c + 1)]

            def planes(t):
                def f(c, hp):
                    return t[hp, P * c : P * (c + 1)]
                return f

            relp, crp, ddp = planes(rel), planes(cr), planes(dd)

            def upp(c, hp):
                return uvp[hp, P * c : P * (c + 1)]

            def spp(c, hp):
                return uvp[hp, P * (3 + c) : P * (4 + c)]

            def cross(out_p, a_p, b_p, hp):
                for c in range(3):
                    c1, c2 = (c + 1) % 3, (c + 2) % 3
                    nc.vector.tensor_mul(tmpA[hp, :], a_p(c1, hp), b_p(c2, hp))
                    nc.vector.tensor_mul(tmpB[hp, :], a_p(c2, hp), b_p(c1, hp))
                    nc.vector.tensor_sub(out_p(c, hp), tmpA[hp, :], tmpB[hp, :])

            def lrp_half(h):
                # lrp = rotate(conj(q), rel) = rel + (2 ux(uxv) - 2w(uxv))/n2
                hp = slice(64 * h, 64 * h + 64)
                cross(crp, lambda c, s: qp(c + 1, s), relp, hp)
                cross(ddp, lambda c, s: qp(c + 1, s), crp, hp)
                for c in range(3):
                    nc.vector.tensor_mul(tmpA[hp, :], qp(0, hp), crp(c, hp))
                    nc.vector.tensor_sub(tmpB[hp, :], ddp(c, hp), tmpA[hp, :])
                    nc.vector.tensor_mul(tmpC[hp, :], tmpB[hp, :], inv2[hp, :])
                    nc.vector.scalar_tensor_tensor(
                        lrp[hp, P * c : P * (c + 1)], tmpC[hp, :], 2.0,
                        relp(c, hp), OP.mult, OP.add,
                    )
                # forward bridge: row c col 8192h+(128(p-64h)+j) = lrp_c[p, j]
                for c in range(6):
                    nc.sync.dma_start(
                        rhsT[c : c + 1, 8192 * h : 8192 * (h + 1)],
                        lrp[hp, P * (c % 3) : P * (c % 3 + 1)],
                    )

            def l1_half(h, h1s):
                # layer 1 (K=8), groups g = 16h + 4*sig + bb
                for bb in range(4):
                    for sig in range(4):
                        g = 16 * h + 4 * sig + bb
                        rhs_g = rhsT[:, 512 * g : 512 * (g + 1)]
                        h1 = h1p.tile([P, 1024], BF16, tag="h1", name="h1")
                        p1 = ps1.tile([P, 1024], F32, tag="p1", name="p1")
                        for fc in range(2):
                            nc.tensor.matmul(
                                p1[:, 512 * fc : 512 * (fc + 1)],
                                lhsT_all[:, 128 * (2 * g + fc) : 128 * (2 * g + fc) + 128],
                                rhs_g,
                                start=True, stop=True,
                            )
                        nc.scalar.activation(h1[:], p1[:], GELU)
                        h1s[g] = h1

            def l23_half(h, h1s):
                # layers 2+3 (K=128); psum3 block beta = 4h+bb packs 4 sigs
                for bb in range(4):
                    beta = 4 * h + bb
                    for sig in range(4):
                        g = 16 * h + 4 * sig + bb
                        h1 = h1s[g]
                        p2 = ps2.tile([P, 512], F32, tag="p2", name="p2")
                        for kc in range(2):
                            nc.tensor.matmul(
                                p2[:],
                                w2[:, 128 * kc : 128 * (kc + 1)],
                                h1[:, 512 * kc : 512 * (kc + 1)],
                                start=(kc == 0), stop=(kc == 1),
                            )
                        h2 = actp.tile([P, 512], BF16, tag="h2", name="h2")
                        nc.scalar.activation(h2[:], p2[:], GELU, bias=b2t[:, 0:1])
                        if sig == 0:
                            p3b = ps3.tile([P, 512], F32, tag="p3", name="p3")
                        nc.tensor.matmul(
                            p3b[32 * sig : 32 * sig + 32, :],
                            wtr[:],
                            h2[:],
                            start=True, stop=True,
                            tile_position=(0, 32 * sig),
                        )
                        if sig == 3:
                            nc.vector.tensor_scalar_add(
                                uvT[:, 512 * beta : 512 * (beta + 1)],
                                p3b[:], btr[:, 0:1],
                            )

            def out_half(h):
                hp = slice(64 * h, 64 * h + 64)
                # reverse bridge: uvT[32sig+k, 2048h+512bb+128q+j] ->
                # uvp[64h+16sig+4bb+q, 128k+j]; one DMA per k (4 sig strips)
                for k in range(6):
                    nc.sync.dma_start(
                        uvp[hp, P * k : P * (k + 1)],
                        uvT[k : k + 97 : 32, 2048 * h : 2048 * (h + 1)],
                    )
                # trans_vel = rotate(q, u) = u + (2 ux(uxu') + 2w(uxv))/n2
                cross(crp, lambda c, s: qp(c + 1, s), upp, hp)
                cross(ddp, lambda c, s: qp(c + 1, s), crp, hp)
                for c in range(3):
                    nc.vector.tensor_mul(tmpA[hp, :], qp(0, hp), crp(c, hp))
                    nc.vector.tensor_add(tmpB[hp, :], ddp(c, hp), tmpA[hp, :])
                    nc.vector.tensor_mul(tmpC[hp, :], tmpB[hp, :], inv2[hp, :])
                    nc.vector.scalar_tensor_tensor(
                        otile[hp, (4 + c)::7], tmpC[hp, :], 2.0, upp(c, hp),
                        OP.mult, OP.add,
                    )
                # quat_vel = quat_mult(q_raw, (0, s)),  s = 0.05*(h2@Wr+br)
                qw = lambda: qp(0, hp)
                qv = [qp(1, hp), qp(2, hp), qp(3, hp)]
                # w: -(qx s0 + qy s1 + qz s2)
                nc.vector.tensor_mul(tmpA[hp, :], qv[0], spp(0, hp))
                nc.vector.tensor_mul(tmpB[hp, :], qv[1], spp(1, hp))
                nc.vector.tensor_add(tmpC[hp, :], tmpA[hp, :], tmpB[hp, :])
                nc.vector.tensor_mul(tmpA[hp, :], qv[2], spp(2, hp))
                nc.vector.scalar_tensor_tensor(
                    otile[hp, 0::7], tmpA[hp, :], -1.0, tmpC[hp, :],
                    OP.mult, OP.subtract,
                )
                # xyz: qw s_c + (q_{c+1} s_{c+2} - q_{c+2} s_{c+1})
                for c in range(3):
                    c1, c2 = (c + 1) % 3, (c + 2) % 3
                    nc.vector.tensor_mul(tmpA[hp, :], qv[c1], spp(c2, hp))
                    nc.vector.tensor_mul(tmpB[hp, :], qv[c2], spp(c1, hp))
                    nc.vector.tensor_sub(tmpC[hp, :], tmpA[hp, :], tmpB[hp, :])
                    nc.vector.tensor_mul(tmpA[hp, :], qw(), spp(c, hp))
                    nc.vector.tensor_add(
                        otile[hp, (1 + c)::7], tmpA[hp, :], tmpC[hp, :])
                nc.sync.dma_start(out_d[hp, :], otile[hp, :])

            # ---------- PE warm-up ----------
            # The HAM clock gate needs ~3.4us of sustained PE activity to
            # lift the PE from 1.2 to 2.4 GHz, and re-throttles after ~3.4us
            # idle.  Dummy matmuls bridge the DVE-only head so the real
            # pipeline starts (and stays) warm.
            for w in range(14):
                pw = ps2.tile([P, 512], F32, tag="p2", name="warm")
                nc.tensor.matmul(pw[:], g128[:], qt[:, 0:512],
                                 start=True, stop=True)

            # ---------- schedule ----------
            h1s = {}
            lrp_half(0)
            # trackers: keep the PE busy while the lrp-h0 tail + bridge run
            for rng in ((0, 256), (256, 384)):
                pw = ps2.tile([P, 512], F32, tag="p2", name="warm")
                nc.tensor.matmul(pw[:, 0 : rng[1] - rng[0]],
                                 w2[0:64, 0:128],
                                 lrp[0:64, rng[0] : rng[1]],
                                 start=True, stop=True)
            l1_half(0, h1s)
            l23_half(0, h1s)
            lrp_half(1)          # overlaps half-0 matmuls/gelu
            l1_half(1, h1s)
            out_half(0)          # overlaps half-1 matmuls/gelu
            l23_half(1, h1s)
            out_half(1)

    nc.finalize()
    return nc


def make_in_maps(scalar_features, quat, trans, W1, b1, W2, b2, Wt, bt, Wr, br):
    import ml_dtypes
    f32 = np.float32
    bf16 = ml_dtypes.bfloat16
    sf = np.asarray(scalar_features, f32).reshape(PAIRS, D)
    qf = np.asarray(quat, f32).reshape(PAIRS * R * 4)
    tf = np.asarray(trans, f32).reshape(PAIRS * R * 3)
    W1 = np.asarray(W1, f32)
    W1a = np.ascontiguousarray(W1[:D])
    W1b = np.ascontiguousarray(W1[D:])                     # [3, 256]
    W1b_hi = W1b.astype(bf16)
    W1b_lo = (W1b - W1b_hi.astype(f32)).astype(bf16)
    W1bc = np.zeros((8, 8192), bf16)
    for g in range(PPC):
        for fc in range(2):
            col = 128 * (2 * g + fc)
            W1bc[0:3, col : col + 128] = W1b_hi[:, 128 * fc : 128 * (fc + 1)]
            W1bc[3:6, col : col + 128] = W1b_lo[:, 128 * fc : 128 * (fc + 1)]
    W1aE = np.concatenate([W1a, np.asarray(b1, f32).reshape(1, D)], axis=0)
    W2 = np.ascontiguousarray(np.asarray(W2, f32)).astype(bf16)
    b2t = np.asarray(b2, f32).reshape(128, 1)
    Wtr = np.zeros((128, 32), f32)
    Wtr[:, 0:3] = np.asarray(Wt, f32)
    Wtr[:, 3:6] = 0.05 * np.asarray(Wr, f32)
    Wtr = Wtr.astype(bf16)
    btr = np.zeros((P, 1), f32)
    for m in range(4):
        btr[32 * m : 32 * m + 3, 0] = np.asarray(bt, f32)
        btr[32 * m + 3 : 32 * m + 6, 0] = 0.05 * np.asarray(br, f32)
    G = np.kron(np.eye(32, dtype=f32), np.ones((4, 4), f32))
    ones2 = np.ones((2, 16384), bf16)

    in_maps = []
    for i in range(NCORES):
        sl = slice(PPC * i, PPC * (i + 1))
        sfTe = np.concatenate(
            [np.ascontiguousarray(sf[sl].T), np.ones((1, PPC), f32)], axis=0)
        in_maps.append({
            "quat": np.ascontiguousarray(
                qf[TOK * 4 * i : TOK * 4 * (i + 1)].reshape(P, 512)),
            "trans": np.ascontiguousarray(
                tf[TOK * 3 * i : TOK * 3 * (i + 1)].reshape(P, 384)),
            "sfTe": sfTe, "W1aE": W1aE, "W1bc": W1bc, "ones2": ones2,
            "W2": W2, "b2t": b2t,
            "Wtr": Wtr, "btr": btr, "G": G,
        })
    return in_maps


_NC_CACHE = None


def kernel(**inputs):
    global _NC_CACHE
    if _NC_CACHE is None:
        _NC_CACHE = build_nc()
    in_maps = make_in_maps(**inputs)
    res = run_bass_kernel_spmd(_NC_CACHE, in_maps, list(range(NCORES))).results
    outs = [res[i]["out"].reshape(TOK, 7) for i in range(NCORES)]
    return np.concatenate(outs, axis=0).reshape(B, T, R, 7)


if __name__ == "__main__":
    rng = np.random.default_rng(0)
    ins = {
        "scalar_features": rng.standard_normal((B, T, D), dtype=np.float32),
        "quat": rng.standard_normal((B, T, R, 4), dtype=np.float32),
        "trans": rng.standard_normal((B, T, R, 3), dtype=np.float32),
        "W1": rng.standard_normal((D + 3, D), dtype=np.float32) * 0.06,
        "b1": np.zeros(D, np.float32),
        "W2": rng.standard_normal((D, D // 2), dtype=np.float32) * 0.06,
        "b2": np.zeros(D // 2, np.float32),
        "Wt": rng.standard_normal((D // 2, 3), dtype=np.float32) * 0.09,
        "bt": np.zeros(3, np.float32),
        "Wr": rng.standard_normal((D // 2, 3), dtype=np.float32) * 0.09,
        "br": np.zeros(3, np.float32),
    }
    out = kernel(**ins)
    print("kernel output shape:", out.shape)


# revision 18
# speedup vs baseline: 1.0890x; 1.0890x over previous
"""Trainium2 Bass kernel for nn_EquivariantOutputHead.

Reference computation (B=8, T=32, R=512, D=256):
  x    = broadcast(scalar_features)                      (B,T,R,D)
  rel  = trans - mean_R(trans)
  lrp  = rotate(conj(normalize(quat)), rel)
  h1   = gelu([x, lrp] @ W1 + b1)
  h2   = gelu(h1 @ W2 + b2)
  tv   = rotate(normalize(quat), h2 @ Wt + bt)
  qv   = 0.5 * quat_mult(quat, (0, 0.1*(h2 @ Wr + br)))
  out  = [qv, tv]                                        (B,T,R,7)

Sharding: data-parallel over the 256 (b,t) pairs -> 32 pairs (16384 tokens)
per core.  sf @ W1[:D] + b1 is computed once per (b,t) (tiny matmul) and
folded into layer 1 as two bf16 hi/lo contraction rows (rhs rows = ones);
the layer-1 matmul is K=8 (3 lrp hi + 3 lrp lo + 2 c rows).

Rotations use v' = v +/- 2w(uxv)/|q|^2 + 2ux(uxv)/|q|^2 with raw quat
components - only 1/n2 (DVE reciprocal) is needed, no sqrt table.

The 32 groups are processed in two 16-group halves; all plane (DVE) work
for a half touches only its 64 partitions, so half-1 preprocessing and
half-0 output rotation overlap the matmul/gelu window of the other half.
"""

import os
import sys

for _p in ("/opt/trn_rl_repo",):
    if _p not in sys.path:
        sys.path.insert(0, _p)

import numpy as np

import concourse.bacc as bacc
import concourse.bass as bass
import concourse.mybir as mybir
import concourse.tile as tile
from concourse.bass_utils import run_bass_kernel_spmd

F32 = mybir.dt.float32
BF16 = mybir.dt.bfloat16
AF = mybir.ActivationFunctionType
OP = mybir.AluOpType
AX = mybir.AxisListType

B, T, R, D = 8, 32, 512, 256
NCORES = 8
PAIRS = B * T              # 256 (b,t) pairs
PPC = PAIRS // NCORES      # 32 pairs (groups) per core
TOK = PPC * R              # 16384 tokens per core
P = 128                    # partitions
NBLK = 8                   # uvT blocks (4 groups each)

GELU = AF.Gelu_apprx_tanh


def build_nc():
    nc = bacc.Bacc(None)

    quat_d = nc.declare_dram_parameter("quat", [P, 512], F32, isOutput=False)
    trans_d = nc.declare_dram_parameter("trans", [P, 384], F32, isOutput=False)
    sfTe_d = nc.declare_dram_parameter("sfTe", [257, PPC], F32, isOutput=False)
    w1aE_d = nc.declare_dram_parameter("W1aE", [257, 256], F32, isOutput=False)
    w1bc_d = nc.declare_dram_parameter("W1bc", [8, 8192], BF16, isOutput=False)
    ones_d = nc.declare_dram_parameter("ones2", [2, 16384], BF16, isOutput=False)
    w2_d = nc.declare_dram_parameter("W2", [256, 128], BF16, isOutput=False)
    b2t_d = nc.declare_dram_parameter("b2t", [P, 1], F32, isOutput=False)
    wtr_d = nc.declare_dram_parameter("Wtr", [P, 32], BF16, isOutput=False)
    btr_d = nc.declare_dram_parameter("btr", [P, 1], F32, isOutput=False)
    g_d = nc.declare_dram_parameter("G", [P, P], F32, isOutput=False)
    out_d = nc.declare_dram_parameter("out", [P, 896], F32, isOutput=True)

    with tile.TileContext(nc) as tc:
        with (
            tc.tile_pool(name="main", bufs=1) as main,
            tc.tile_pool(name="act", bufs=4) as actp,
            tc.tile_pool(name="h1p", bufs=18) as h1p,
            tc.tile_pool(name="ps1", bufs=2, space="PSUM") as ps1,
            tc.tile_pool(name="ps2", bufs=3, space="PSUM") as ps2,
            tc.tile_pool(name="ps3", bufs=1, space="PSUM") as ps3,
        ):
            # ---------- persistent SBUF tensors ----------
            qt = main.tile([P, 512], F32, tag="qt")     # raw quat, interleaved
            tt = main.tile([P, 384], F32, tag="tt")     # trans, interleaved
            sfTe = main.tile([P, 3 * PPC], F32, tag="sfTe")
            w1aE = main.tile([P, 768], F32, tag="w1aE")
            lhsT_all = main.tile([8, 8192], BF16, tag="lhsT_all")
            w2 = main.tile([P, 256], BF16, tag="w2")
            b2t = main.tile([P, 1], F32, tag="b2t")
            wtr = main.tile([P, 32], BF16, tag="wtr")
            btr = main.tile([P, 1], F32, tag="btr")
            g128 = main.tile([P, P], F32, tag="g128")

            cRhi = main.tile([PPC, 256], BF16, tag="cRhi")
            cRhf = main.tile([PPC, 256], F32, tag="cRhf")
            cRlo = main.tile([PPC, 256], BF16, tag="cRlo")
            S3 = main.tile([P, 3], F32, tag="S3")
            cent = main.tile([P, 3], F32, tag="cent")
            qc = main.tile([P, 512], F32, tag="qc")      # compact quat planes
            n2 = main.tile([P, P], F32, tag="n2")
            # duplicated-plane layouts: [x y z x y] etc. so cross products
            # batch as single [*,384] DVE ops over 3 contiguous planes
            qcd = main.tile([P, 640], F32, tag="qcd")    # vec planes dup
            wd = main.tile([P, 384], F32, tag="wd")      # w plane x3
            inv2d = main.tile([P, 384], F32, tag="inv2d")  # 1/|q|^2 x3
            reld = main.tile([P, 640], F32, tag="reld")  # rel planes dup
            crd = main.tile([P, 640], F32, tag="crd")    # cross dup
            dd = main.tile([P, 384], F32, tag="dd")
            lrp = main.tile([P, 384], BF16, tag="lrp")
            tmpA = main.tile([P, P], F32, tag="tmpA")
            tmpD = main.tile([P, 384], F32, tag="tmpD")
            tmpE = main.tile([P, 384], F32, tag="tmpE")
            rhsT = main.tile([8, 16384], BF16, tag="rhsT")
            uvT = main.tile([P, 512 * NBLK], F32, tag="uvT")
            # uvp planes: u(0:384) udup(384:640) s(640:1024) sdup(1024:1280)
            uvp = main.tile([P, 1280], F32, tag="uvp")
            otile = main.tile([P, 896], F32, tag="otile")

            # ---------- loads (sync HWDGE; scalar stays pure-ACT) ----------
            # order = need time: qt/tt/g128 gate the DVE chain + warm-ups
            nc.sync.dma_start(qt[:], quat_d[:])
            nc.sync.dma_start(tt[:], trans_d[:])
            nc.sync.dma_start(g128[:], g_d[:])
            nc.sync.dma_start(sfTe[:, 0:PPC], sfTe_d[0:128, :])
            nc.sync.dma_start(sfTe[:, PPC : 2 * PPC], sfTe_d[128:256, :])
            nc.sync.dma_start(sfTe[0:1, 2 * PPC : 3 * PPC], sfTe_d[256:257, :])
            nc.sync.dma_start(w1aE[:, 0:256], w1aE_d[0:128, :])
            nc.sync.dma_start(w1aE[:, 256:512], w1aE_d[128:256, :])
            nc.sync.dma_start(w1aE[0:1, 512:768], w1aE_d[256:257, :])
            nc.sync.dma_start(lhsT_all[:], w1bc_d[:])
            nc.sync.dma_start(rhsT[6:8, :], ones_d[:])
            nc.sync.dma_start(w2[:, 0:128], w2_d[0:128, :])
            nc.sync.dma_start(w2[:, 128:256], w2_d[128:256, :])
            nc.sync.dma_start(b2t[:], b2t_d[:])
            nc.sync.dma_start(wtr[:], wtr_d[:])
            nc.sync.dma_start(btr[:], btr_d[:])

            # preload the gelu table set off the critical path
            nc.scalar.activation(tmpA[0:1, 0:1], g128[0:1, 0:1], GELU)

            # ---------- quat / centroid prep (full planes) ----------
            # n2 = |q|^2 via square + inner-axis reduce (2 ops)
            nc.vector.tensor_mul(qc[:], qt[:], qt[:])     # scratch: q^2
            nc.vector.reduce_sum(
                n2[:], qc[:].rearrange("p (j c) -> p j c", c=4), axis=AX.X)
            nc.vector.reciprocal(inv2d[:, 0:128], n2[:])
            nc.vector.tensor_copy(inv2d[:, 128:256], inv2d[:, 0:128])
            nc.vector.tensor_copy(inv2d[:, 256:384], inv2d[:, 0:128])
            for c in range(4):
                nc.vector.tensor_copy(qc[:, P * c : P * (c + 1)], qt[:, c::4])
            nc.vector.tensor_copy(qcd[:, 0:384], qc[:, 128:512])
            nc.vector.tensor_copy(qcd[:, 384:640], qc[:, 128:384])
            nc.vector.tensor_copy(wd[:, 0:128], qc[:, 0:128])
            nc.vector.tensor_copy(wd[:, 128:256], qc[:, 0:128])
            nc.vector.tensor_copy(wd[:, 256:384], qc[:, 0:128])
            nc.vector.reduce_sum(
                S3[:, 0:3], tt[:].rearrange("p (j c) -> p c j", c=3), axis=AX.X)
            for h in range(2):
                hp = slice(64 * h, 64 * h + 64)
                psch = ps2.tile([P, 3], F32, tag="p2", name="psc")
                nc.tensor.matmul(psch[hp, :], g128[hp, hp], S3[hp, :],
                                 start=True, stop=True,
                                 tile_position=(64 * h, 64 * h))
                nc.vector.tensor_scalar_mul(cent[hp, :], psch[hp, :], 1.0 / 512.0)
            for c in range(3):
                nc.vector.tensor_scalar_sub(
                    reld[:, P * c : P * (c + 1)], tt[:, c::3], cent[:, c : c + 1]
                )
            nc.vector.tensor_copy(reld[:, 384:640], reld[:, 0:256])

            def cross_batched(out_ap, a_dup, b_dup, hp):
                # out = a x b over 3 planes: a_dup/b_dup are [*,640] dup tiles
                nc.vector.tensor_mul(tmpD[hp, :], a_dup[hp, 128:512],
                                     b_dup[hp, 256:640])
                nc.vector.tensor_mul(tmpE[hp, :], a_dup[hp, 256:640],
                                     b_dup[hp, 128:512])
                nc.vector.tensor_sub(out_ap, tmpD[hp, :], tmpE[hp, :])

            def lrp_full():
                # lrp = rotate(conj(q), rel) = rel + (2 ux(uxv) - 2w(uxv))/n2
                hp = slice(0, 128)
                cross_batched(crd[:, 0:384], qcd, reld, hp)
                nc.vector.tensor_copy(crd[:, 384:640], crd[:, 0:256])
                cross_batched(dd[:, 0:384], qcd, crd, hp)
                nc.vector.tensor_mul(tmpD[:], wd[:], crd[:, 0:384])
                nc.vector.tensor_sub(tmpE[:], dd[:], tmpD[:])
                nc.vector.tensor_mul(tmpD[:], tmpE[:], inv2d[:])
                nc.vector.scalar_tensor_tensor(
                    lrp[:], tmpD[:], 2.0, reld[:, 0:384], OP.mult, OP.add)
                # forward bridge: row c col 128p+j = lrp_c[p, j]
                for c in range(6):
                    nc.sync.dma_start(
                        rhsT[c : c + 1, :],
                        lrp[:, P * (c % 3) : P * (c % 3 + 1)],
                    )

            def l1_half(h, h1s):
                # layer 1 (K=8), groups g = 16h + 4*sig + bb
                for bb in range(4):
                    for sig in range(4):
                        g = 16 * h + 4 * sig + bb
                        rhs_g = rhsT[:, 512 * g : 512 * (g + 1)]
                        h1 = h1p.tile([P, 1024], BF16, tag="h1", name="h1")
                        p1 = ps1.tile([P, 1024], F32, tag="p1", name="p1")
                        for fc in range(2):
                            nc.tensor.matmul(
                                p1[:, 512 * fc : 512 * (fc + 1)],
                                lhsT_all[:, 128 * (2 * g + fc) : 128 * (2 * g + fc) + 128],
                                rhs_g,
                                start=True, stop=True,
                            )
                        nc.scalar.activation(h1[:], p1[:], GELU)
                        h1s[g] = h1

            def l23_half(h, h1s):
                # layers 2+3 (K=128); psum3 block beta = 4h+bb packs 4 sigs
                for bb in range(4):
                    beta = 4 * h + bb
                    for sig in range(4):
                        g = 16 * h + 4 * sig + bb
                        h1 = h1s[g]
                        p2 = ps2.tile([P, 512], F32, tag="p2", name="p2")
                        for kc in range(2):
                            nc.tensor.matmul(
                                p2[:],
                                w2[:, 128 * kc : 128 * (kc + 1)],
                                h1[:, 512 * kc : 512 * (kc + 1)],
                                start=(kc == 0), stop=(kc == 1),
                            )
                        h2 = actp.tile([P, 512], BF16, tag="h2", name="h2")
                        nc.scalar.activation(h2[:], p2[:], GELU, bias=b2t[:, 0:1])
                        if sig == 0:
                            p3b = ps3.tile([P, 512], F32, tag="p3", name="p3")
                        nc.tensor.matmul(
                            p3b[32 * sig : 32 * sig + 32, :],
                            wtr[:],
                            h2[:],
                            start=True, stop=True,
                            tile_position=(0, 32 * sig),
                        )
                        if sig == 3:
                            nc.vector.tensor_scalar_add(
                                uvT[:, 512 * beta : 512 * (beta + 1)],
                                p3b[:], btr[:, 0:1],
                            )

            def out_half(h):
                hp = slice(64 * h, 64 * h + 64)
                # reverse bridge: uvT[32sig+k, 2048h+512bb+128q+j] ->
                # uvp[64h+16sig+4bb+q, ...]; one DMA per k (4 sig strips).
                # u -> planes 0-2, s -> planes 5-7 (cols 640:1024)
                for k in range(6):
                    dst = P * k if k < 3 else P * k + 256
                    nc.sync.dma_start(
                        uvp[hp, dst : dst + P],
                        uvT[k : k + 97 : 32, 2048 * h : 2048 * (h + 1)],
                    )
                nc.vector.tensor_copy(uvp[hp, 384:640], uvp[hp, 0:256])
                nc.vector.tensor_copy(uvp[hp, 1024:1280], uvp[hp, 640:896])
                # trans_vel = rotate(q, u) = u + (2 ux(uxu) + 2w(uxu))/n2
                cross_batched(crd[hp, 0:384], qcd, uvp, hp)
                nc.vector.tensor_copy(crd[hp, 384:640], crd[hp, 0:256])
                cross_batched(dd[hp, 0:384], qcd, crd, hp)
                nc.vector.tensor_mul(tmpD[hp, :], wd[hp, :], crd[hp, 0:384])
                nc.vector.tensor_add(tmpE[hp, :], dd[hp, :], tmpD[hp, :])
                nc.vector.tensor_mul(tmpD[hp, :], tmpE[hp, :], inv2d[hp, :])
                def pl3(t, lo):
                    return t[hp, lo : lo + 384].rearrange(
                        "p (c j) -> p c j", c=3)

                otv = otile[hp, :].rearrange("p (t s) -> p s t", s=7)
                nc.vector.scalar_tensor_tensor(
                    otv[:, 4:7, :], pl3(tmpD, 0), 2.0, pl3(uvp, 0),
                    OP.mult, OP.add,
                )
                # quat_vel = quat_mult(q_raw, (0, s)),  s = 0.05*(h2@Wr+br)
                # w: -(u . s) via elementwise mul + inner-plane reduce
                nc.vector.tensor_mul(tmpD[hp, :], qcd[hp, 0:384],
                                     uvp[hp, 640:1024])
                nc.vector.reduce_sum(
                    tmpA[hp, :],
                    tmpD[hp, :].rearrange("p (c j) -> p j c", c=3), axis=AX.X)
                nc.vector.tensor_scalar_mul(otile[hp, 0::7], tmpA[hp, :], -1.0)
                # xyz: qw s_c + (q_{c+1} s_{c+2} - q_{c+2} s_{c+1})
                nc.vector.tensor_mul(tmpD[hp, :], qcd[hp, 128:512],
                                     uvp[hp, 896:1280])
                nc.vector.tensor_mul(tmpE[hp, :], qcd[hp, 256:640],
                                     uvp[hp, 768:1152])
                nc.vector.tensor_sub(tmpD[hp, :], tmpD[hp, :], tmpE[hp, :])
                nc.vector.tensor_mul(tmpE[hp, :], wd[hp, :], uvp[hp, 640:1024])
                nc.vector.tensor_add(otv[:, 1:4, :], pl3(tmpE, 0), pl3(tmpD, 0))
                nc.sync.dma_start(out_d[hp, :], otile[hp, :])

            # ---------- PE warm-up ----------
            # The HAM clock gate needs ~3.4us of sustained PE activity to
            # lift the PE from 1.2 to 2.4 GHz, and re-throttles after ~3.4us
            # idle.  Dummy matmuls bridge the DVE-only head; data-dependent
            # "tracker" matmuls then pace the PE along the DVE chain so the
            # real pipeline starts warm.
            def warm_mm(rhs_ap, n):
                pw = ps2.tile([P, 512], F32, tag="p2", name="warm")
                nc.tensor.matmul(pw[:, 0:n], g128[:, 0:128], rhs_ap,
                                 start=True, stop=True)

            for w in range(8):
                warm_mm(qt[:, 0:512], 512)
            warm_mm(n2[:], 128)                    # after |q|^2 reduce
            warm_mm(inv2d[:, 0:384], 384)
            warm_mm(qcd[:, 0:512], 512)            # after compact planes
            warm_mm(reld[:, 0:384], 384)           # after rel

            # ---------- schedule ----------
            h1s = {}
            lrp_full()
            # trackers: keep the PE busy while the lrp tail + bridge run
            for rng in ((0, 128), (128, 256), (256, 384)):
                pw = ps2.tile([P, 512], F32, tag="p2", name="warm")
                nc.tensor.matmul(pw[0:128, 0 : rng[1] - rng[0]],
                                 w2[0:64, 0:128],
                                 lrp[0:64, rng[0] : rng[1]],
                                 start=True, stop=True)

            # ---------- cR = sf @ W1a + b1, token-major [32, 256] ----------
            # (emitted after lrp-h0 so its PSUM->SBUF copies don't block the
            # DVE chain; only needed by the time layer 1 starts)
            psc2 = ps2.tile([PPC, 256], F32, tag="p2", name="psc2")
            nc.tensor.matmul(psc2[:], sfTe[:, 0:PPC], w1aE[:, 0:256],
                             start=True, stop=False)
            nc.tensor.matmul(psc2[:], sfTe[:, PPC : 2 * PPC], w1aE[:, 256:512],
                             start=False, stop=False)
            nc.tensor.matmul(psc2[:], sfTe[0:1, 2 * PPC : 3 * PPC],
                             w1aE[0:1, 512:768], start=False, stop=True)
            nc.vector.tensor_copy(cRhi[:], psc2[:])
            nc.vector.tensor_copy(cRhf[:], cRhi[:])
            nc.vector.tensor_sub(cRlo[:], psc2[:], cRhf[:])
            nc.sync.dma_start(lhsT_all[6:7, :], cRhi[:])
            nc.sync.dma_start(lhsT_all[7:8, :], cRlo[:])

            l1_half(0, h1s)
            l23_half(0, h1s)
            l1_half(1, h1s)
            out_half(0)          # overlaps half-1 matmuls/gelu
            l23_half(1, h1s)
            out_half(1)

    nc.finalize()
    return nc


def make_in_maps(scalar_features, quat, trans, W1, b1, W2, b2, Wt, bt, Wr, br):
    import ml_dtypes
    f32 = np.float32
    bf16 = ml_dtypes.bfloat16
    sf = np.asarray(scalar_features, f32).reshape(PAIRS, D)
    qf = np.asarray(quat, f32).reshape(PAIRS * R * 4)
    tf = np.asarray(trans, f32).reshape(PAIRS * R * 3)
    W1 = np.asarray(W1, f32)
    W1a = np.ascontiguousarray(W1[:D])
    W1b = np.ascontiguousarray(W1[D:])                     # [3, 256]
    W1b_hi = W1b.astype(bf16)
    W1b_lo = (W1b - W1b_hi.astype(f32)).astype(bf16)
    W1bc = np.zeros((8, 8192), bf16)
    for g in range(PPC):
        for fc in range(2):
            col = 128 * (2 * g + fc)
            W1bc[0:3, col : col + 128] = W1b_hi[:, 128 * fc : 128 * (fc + 1)]
            W1bc[3:6, col : col + 128] = W1b_lo[:, 128 * fc : 128 * (fc + 1)]
    W1aE = np.concatenate([W1a, np.asarray(b1, f32).reshape(1, D)], axis=0)
    W2 = np.ascontiguousarray(np.asarray(W2, f32)).astype(bf16)
    b2t = np.asarray(b2, f32).reshape(128, 1)
    Wtr = np.zeros((128, 32), f32)
    Wtr[:, 0:3] = np.asarray(Wt, f32)
    Wtr[:, 3:6] = 0.05 * np.asarray(Wr, f32)
    Wtr = Wtr.astype(bf16)
    btr = np.zeros((P, 1), f32)
    for m in range(4):
        btr[32 * m : 32 * m + 3, 0] = np.asarray(bt, f32)
        btr[32 * m + 3 : 32 * m + 6, 0] = 0.05 * np.asarray(br, f32)
    G = np.kron(np.eye(32, dtype=f32), np.ones((4, 4), f32))
    ones2 = np.ones((2, 16384), bf16)

    in_maps = []
    for i in range(NCORES):
        sl = slice(PPC * i, PPC * (i + 1))
        sfTe = np.concatenate(
            [np.ascontiguousarray(sf[sl].T), np.ones((1, PPC), f32)], axis=0)
        in_maps.append({
            "quat": np.ascontiguousarray(
                qf[TOK * 4 * i : TOK * 4 * (i + 1)].reshape(P, 512)),
            "trans": np.ascontiguousarray(
                tf[TOK * 3 * i : TOK * 3 * (i + 1)].reshape(P, 384)),
            "sfTe": sfTe, "W1aE": W1aE, "W1bc": W1bc, "ones2": ones2,
            "W2": W2, "b2t": b2t,
            "Wtr": Wtr, "btr": btr, "G": G,
        })
    return in_maps


_NC_CACHE = None


def kernel(**inputs):
    global _NC_CACHE
    if _NC_CACHE is None:
        _NC_CACHE = build_nc()
    in_maps = make_in_maps(**inputs)
    res = run_bass_kernel_spmd(_NC_CACHE, in_maps, list(range(NCORES))).results
    outs = [res[i]["out"].reshape(TOK, 7) for i in range(NCORES)]
    return np.concatenate(outs, axis=0).reshape(B, T, R, 7)


if __name__ == "__main__":
    rng = np.random.default_rng(0)
    ins = {
        "scalar_features": rng.standard_normal((B, T, D), dtype=np.float32),
        "quat": rng.standard_normal((B, T, R, 4), dtype=np.float32),
        "trans": rng.standard_normal((B, T, R, 3), dtype=np.float32),
        "W1": rng.standard_normal((D + 3, D), dtype=np.float32) * 0.06,
        "b1": np.zeros(D, np.float32),
        "W2": rng.standard_normal((D, D // 2), dtype=np.float32) * 0.06,
        "b2": np.zeros(D // 2, np.float32),
        "Wt": rng.standard_normal((D // 2, 3), dtype=np.float32) * 0.09,
        "bt": np.zeros(3, np.float32),
        "Wr": rng.standard_normal((D // 2, 3), dtype=np.float32) * 0.09,
        "br": np.zeros(3, np.float32),
    }
    out = kernel(**ins)
    print("kernel output shape:", out.shape)


# revision 23
# speedup vs baseline: 1.0947x; 1.0053x over previous
"""Trainium2 Bass kernel for nn_EquivariantOutputHead.

Reference computation (B=8, T=32, R=512, D=256):
  x    = broadcast(scalar_features)                      (B,T,R,D)
  rel  = trans - mean_R(trans)
  lrp  = rotate(conj(normalize(quat)), rel)
  h1   = gelu([x, lrp] @ W1 + b1)
  h2   = gelu(h1 @ W2 + b2)
  tv   = rotate(normalize(quat), h2 @ Wt + bt)
  qv   = 0.5 * quat_mult(quat, (0, 0.1*(h2 @ Wr + br)))
  out  = [qv, tv]                                        (B,T,R,7)

Sharding: data-parallel over the 256 (b,t) pairs -> 32 pairs (16384 tokens)
per core.  sf @ W1[:D] + b1 is computed once per (b,t) (tiny matmul) and
folded into layer 1 as two bf16 hi/lo contraction rows (rhs rows = ones);
the layer-1 matmul is K=8 (3 lrp hi + 3 lrp lo + 2 c rows).

Rotations use v' = v +/- 2w(uxv)/|q|^2 + 2ux(uxv)/|q|^2 with raw quat
components - only 1/n2 (DVE reciprocal) is needed, no sqrt table.

The 32 groups are processed in two 16-group halves; all plane (DVE) work
for a half touches only its 64 partitions, so half-1 preprocessing and
half-0 output rotation overlap the matmul/gelu window of the other half.
"""

import os
import sys

for _p in ("/opt/trn_rl_repo",):
    if _p not in sys.path:
        sys.path.insert(0, _p)

import numpy as np

import concourse.bacc as bacc
import concourse.bass as bass
import concourse.mybir as mybir
import concourse.tile as tile
from concourse.bass_utils import run_bass_kernel_spmd

F32 = mybir.dt.float32
BF16 = mybir.dt.bfloat16
AF = mybir.ActivationFunctionType
OP = mybir.AluOpType
AX = mybir.AxisListType

B, T, R, D = 8, 32, 512, 256
NCORES = 8
PAIRS = B * T              # 256 (b,t) pairs
PPC = PAIRS // NCORES      # 32 pairs (groups) per core
TOK = PPC * R              # 16384 tokens per core
P = 128                    # partitions
NBLK = 8                   # uvT blocks (4 groups each)

GELU = AF.Gelu_apprx_tanh


def build_nc():
    nc = bacc.Bacc(None)

    quat_d = nc.declare_dram_parameter("quat", [P, 512], F32, isOutput=False)
    trans_d = nc.declare_dram_parameter("trans", [P, 384], F32, isOutput=False)
    sfTe_d = nc.declare_dram_parameter("sfTe", [257, PPC], F32, isOutput=False)
    w1aE_d = nc.declare_dram_parameter("W1aE", [257, 256], F32, isOutput=False)
    w1bc_d = nc.declare_dram_parameter("W1bc", [8, 8192], BF16, isOutput=False)
    ones_d = nc.declare_dram_parameter("ones2", [2, 16384], BF16, isOutput=False)
    w2_d = nc.declare_dram_parameter("W2", [256, 128], BF16, isOutput=False)
    b2t_d = nc.declare_dram_parameter("b2t", [P, 1], F32, isOutput=False)
    wtr_d = nc.declare_dram_parameter("Wtr", [P, 32], BF16, isOutput=False)
    btr_d = nc.declare_dram_parameter("btr", [P, 1], F32, isOutput=False)
    g_d = nc.declare_dram_parameter("G", [P, P], F32, isOutput=False)
    out_d = nc.declare_dram_parameter("out", [P, 896], F32, isOutput=True)

    with tile.TileContext(nc) as tc:
        with (
            tc.tile_pool(name="main", bufs=1) as main,
            tc.tile_pool(name="act", bufs=4) as actp,
            tc.tile_pool(name="h1p", bufs=18) as h1p,
            tc.tile_pool(name="ps1", bufs=2, space="PSUM") as ps1,
            tc.tile_pool(name="ps2", bufs=3, space="PSUM") as ps2,
            tc.tile_pool(name="ps3", bufs=1, space="PSUM") as ps3,
        ):
            # ---------- persistent SBUF tensors ----------
            qt = main.tile([P, 512], F32, tag="qt")     # raw quat, interleaved
            tt = main.tile([P, 384], F32, tag="tt")     # trans, interleaved
            sfTe = main.tile([P, 3 * PPC], F32, tag="sfTe")
            w1aE = main.tile([P, 768], F32, tag="w1aE")
            lhsT_all = main.tile([8, 8192], BF16, tag="lhsT_all")
            w2 = main.tile([P, 256], BF16, tag="w2")
            b2t = main.tile([P, 1], F32, tag="b2t")
            wtr = main.tile([P, 32], BF16, tag="wtr")
            btr = main.tile([P, 1], F32, tag="btr")
            g128 = main.tile([P, P], F32, tag="g128")

            cRhi = main.tile([PPC, 256], BF16, tag="cRhi")
            cRhf = main.tile([PPC, 256], F32, tag="cRhf")
            cRlo = main.tile([PPC, 256], BF16, tag="cRlo")
            S3 = main.tile([P, 3], F32, tag="S3")
            cent = main.tile([P, 3], F32, tag="cent")
            qc = main.tile([P, 512], F32, tag="qc")      # compact quat planes
            n2 = main.tile([P, P], F32, tag="n2")
            # duplicated-plane layouts: [x y z x y] etc. so cross products
            # batch as single [*,384] DVE ops over 3 contiguous planes
            qcd = main.tile([P, 640], F32, tag="qcd")    # vec planes dup
            wd = main.tile([P, 384], F32, tag="wd")      # w plane x3
            inv2d = main.tile([P, 384], F32, tag="inv2d")  # 1/|q|^2 x3
            reld = main.tile([P, 640], F32, tag="reld")  # rel planes dup
            crd = main.tile([P, 640], F32, tag="crd")    # cross dup
            dd = main.tile([P, 384], F32, tag="dd")
            lrp = main.tile([P, 384], BF16, tag="lrp")
            tmpA = main.tile([P, P], F32, tag="tmpA")
            tmpD = main.tile([P, 384], F32, tag="tmpD")
            tmpE = main.tile([P, 384], F32, tag="tmpE")
            rhsT = main.tile([8, 16384], BF16, tag="rhsT")
            uvT = main.tile([P, 512 * NBLK], F32, tag="uvT")
            # uvp planes: u(0:384) udup(384:640) s(640:1024) sdup(1024:1280)
            uvp = main.tile([P, 1280], F32, tag="uvp")
            otile = main.tile([P, 896], F32, tag="otile")

            # ---------- loads (sync HWDGE; scalar stays pure-ACT) ----------
            # order = need time: qt/tt/g128 gate the DVE chain + warm-ups
            nc.sync.dma_start(qt[:], quat_d[:])
            nc.sync.dma_start(sfTe[:, 0:PPC], sfTe_d[0:128, :])
            nc.sync.dma_start(sfTe[:, PPC : 2 * PPC], sfTe_d[128:256, :])
            nc.sync.dma_start(sfTe[0:1, 2 * PPC : 3 * PPC], sfTe_d[256:257, :])
            nc.sync.dma_start(w1aE[:, 0:256], w1aE_d[0:128, :])
            nc.sync.dma_start(w1aE[:, 256:512], w1aE_d[128:256, :])
            nc.sync.dma_start(w1aE[0:1, 512:768], w1aE_d[256:257, :])
            nc.sync.dma_start(tt[:], trans_d[:])
            nc.sync.dma_start(g128[:], g_d[:])
            nc.sync.dma_start(lhsT_all[:], w1bc_d[:])
            nc.sync.dma_start(rhsT[6:8, :], ones_d[:])
            nc.sync.dma_start(w2[:, 0:128], w2_d[0:128, :])
            nc.sync.dma_start(w2[:, 128:256], w2_d[128:256, :])
            nc.sync.dma_start(b2t[:], b2t_d[:])
            nc.sync.dma_start(wtr[:], wtr_d[:])
            nc.sync.dma_start(btr[:], btr_d[:])

            # preload the gelu table set off the critical path
            nc.scalar.activation(tmpA[0:1, 0:1], qt[0:1, 0:1], GELU)

            # ---------- quat prep (full planes; only needs qt) ----------
            # n2 = |q|^2 via square + inner-axis reduce (2 ops)
            nc.vector.tensor_mul(qc[:], qt[:], qt[:])     # scratch: q^2
            nc.vector.reduce_sum(
                n2[:], qc[:].rearrange("p (j c) -> p j c", c=4), axis=AX.X)
            nc.vector.reciprocal(inv2d[:, 0:128], n2[:])
            nc.vector.tensor_copy(inv2d[:, 128:256], inv2d[:, 0:128])
            nc.vector.tensor_copy(inv2d[:, 256:384], inv2d[:, 0:128])
            for c in range(4):
                nc.vector.tensor_copy(qc[:, P * c : P * (c + 1)], qt[:, c::4])
            nc.vector.tensor_copy(qcd[:, 0:384], qc[:, 128:512])
            nc.vector.tensor_copy(qcd[:, 384:640], qc[:, 128:384])
            nc.vector.tensor_copy(wd[:, 0:128], qc[:, 0:128])
            nc.vector.tensor_copy(wd[:, 128:256], qc[:, 0:128])
            nc.vector.tensor_copy(wd[:, 256:384], qc[:, 0:128])

            # ---------- cR = sf @ W1a + b1, token-major [32, 256] ----------
            # early: the scatter DMA completion gates layer 1
            psc2 = ps2.tile([PPC, 256], F32, tag="p2", name="psc2")
            nc.tensor.matmul(psc2[:], sfTe[:, 0:PPC], w1aE[:, 0:256],
                             start=True, stop=False)
            nc.tensor.matmul(psc2[:], sfTe[:, PPC : 2 * PPC], w1aE[:, 256:512],
                             start=False, stop=False)
            nc.tensor.matmul(psc2[:], sfTe[0:1, 2 * PPC : 3 * PPC],
                             w1aE[0:1, 512:768], start=False, stop=True)
            nc.vector.tensor_copy(cRhi[:], psc2[:])
            nc.vector.tensor_copy(cRhf[:], cRhi[:])
            nc.vector.tensor_sub(cRlo[:], psc2[:], cRhf[:])
            nc.sync.dma_start(lhsT_all[6:7, :], cRhi[:])
            nc.sync.dma_start(lhsT_all[7:8, :], cRlo[:])

            # ---------- centroid prep ----------
            nc.vector.reduce_sum(
                S3[:, 0:3], tt[:].rearrange("p (j c) -> p c j", c=3), axis=AX.X)
            for h in range(2):
                hp = slice(64 * h, 64 * h + 64)
                psch = ps2.tile([P, 3], F32, tag="p2", name="psc")
                nc.tensor.matmul(psch[hp, :], g128[hp, hp], S3[hp, :],
                                 start=True, stop=True,
                                 tile_position=(64 * h, 64 * h))
                nc.vector.tensor_scalar_mul(cent[hp, :], psch[hp, :], 1.0 / 512.0)
            for c in range(3):
                nc.vector.tensor_scalar_sub(
                    reld[:, P * c : P * (c + 1)], tt[:, c::3], cent[:, c : c + 1]
                )
            nc.vector.tensor_copy(reld[:, 384:640], reld[:, 0:256])

            def cross_batched(out_ap, a_dup, b_dup, hp):
                # out = a x b over 3 planes: a_dup/b_dup are [*,640] dup tiles
                nc.vector.tensor_mul(tmpD[hp, :], a_dup[hp, 128:512],
                                     b_dup[hp, 256:640])
                nc.vector.tensor_mul(tmpE[hp, :], a_dup[hp, 256:640],
                                     b_dup[hp, 128:512])
                nc.vector.tensor_sub(out_ap, tmpD[hp, :], tmpE[hp, :])

            def lrp_full():
                # lrp = rotate(conj(q), rel) = rel + (2 ux(uxv) - 2w(uxv))/n2
                hp = slice(0, 128)
                cross_batched(crd[:, 0:384], qcd, reld, hp)
                nc.vector.tensor_copy(crd[:, 384:640], crd[:, 0:256])
                cross_batched(dd[:, 0:384], qcd, crd, hp)
                nc.vector.tensor_mul(tmpD[:], wd[:], crd[:, 0:384])
                nc.vector.tensor_sub(tmpE[:], dd[:], tmpD[:])
                nc.vector.tensor_mul(tmpD[:], tmpE[:], inv2d[:])
                nc.vector.scalar_tensor_tensor(
                    lrp[:], tmpD[:], 2.0, reld[:, 0:384], OP.mult, OP.add)
                # forward bridge: row c col 128p+j = lrp_c[p, j]
                for c in range(6):
                    eng = nc.sync if c < 3 else nc.scalar
                    eng.dma_start(
                        rhsT[c : c + 1, :],
                        lrp[:, P * (c % 3) : P * (c % 3 + 1)],
                    )

            def l1_half(h, h1s):
                # layer 1 (K=8), groups g = 16h + 4*sig + bb
                for bb in range(4):
                    for sig in range(4):
                        g = 16 * h + 4 * sig + bb
                        rhs_g = rhsT[:, 512 * g : 512 * (g + 1)]
                        h1 = h1p.tile([P, 1024], BF16, tag="h1", name="h1")
                        p1 = ps1.tile([P, 1024], F32, tag="p1", name="p1")
                        for fc in range(2):
                            nc.tensor.matmul(
                                p1[:, 512 * fc : 512 * (fc + 1)],
                                lhsT_all[:, 128 * (2 * g + fc) : 128 * (2 * g + fc) + 128],
                                rhs_g,
                                start=True, stop=True,
                            )
                        nc.scalar.activation(h1[:], p1[:], GELU)
                        h1s[g] = h1

            def l23_half(h, h1s):
                # layers 2+3 (K=128); psum3 block beta = 4h+bb packs 4 sigs
                for bb in range(4):
                    beta = 4 * h + bb
                    for sig in range(4):
                        g = 16 * h + 4 * sig + bb
                        h1 = h1s[g]
                        p2 = ps2.tile([P, 512], F32, tag="p2", name="p2")
                        for kc in range(2):
                            nc.tensor.matmul(
                                p2[:],
                                w2[:, 128 * kc : 128 * (kc + 1)],
                                h1[:, 512 * kc : 512 * (kc + 1)],
                                start=(kc == 0), stop=(kc == 1),
                            )
                        h2 = actp.tile([P, 512], BF16, tag="h2", name="h2")
                        nc.scalar.activation(h2[:], p2[:], GELU, bias=b2t[:, 0:1])
                        if sig == 0:
                            p3b = ps3.tile([P, 512], F32, tag="p3", name="p3")
                        nc.tensor.matmul(
                            p3b[32 * sig : 32 * sig + 32, :],
                            wtr[:],
                            h2[:],
                            start=True, stop=True,
                            tile_position=(0, 32 * sig),
                        )
                        if sig == 3:
                            nc.vector.tensor_scalar_add(
                                uvT[:, 512 * beta : 512 * (beta + 1)],
                                p3b[:], btr[:, 0:1],
                            )

            def out_half(h):
                hp = slice(64 * h, 64 * h + 64)
                # reverse bridge: uvT[32sig+k, 2048h+512bb+128q+j] ->
                # uvp[64h+16sig+4bb+q, ...]; one DMA per k (4 sig strips).
                # u -> planes 0-2, s -> planes 5-7 (cols 640:1024)
                for k in range(6):
                    dst = P * k if k < 3 else P * k + 256
                    nc.sync.dma_start(
                        uvp[hp, dst : dst + P],
                        uvT[k : k + 97 : 32, 2048 * h : 2048 * (h + 1)],
                    )
                nc.vector.tensor_copy(uvp[hp, 384:640], uvp[hp, 0:256])
                nc.vector.tensor_copy(uvp[hp, 1024:1280], uvp[hp, 640:896])
                # trans_vel = rotate(q, u) = u + (2 ux(uxu) + 2w(uxu))/n2
                cross_batched(crd[hp, 0:384], qcd, uvp, hp)
                nc.vector.tensor_copy(crd[hp, 384:640], crd[hp, 0:256])
                cross_batched(dd[hp, 0:384], qcd, crd, hp)
                nc.vector.tensor_mul(tmpD[hp, :], wd[hp, :], crd[hp, 0:384])
                nc.vector.tensor_add(tmpE[hp, :], dd[hp, :], tmpD[hp, :])
                nc.vector.tensor_mul(tmpD[hp, :], tmpE[hp, :], inv2d[hp, :])
                def pl3(t, lo):
                    return t[hp, lo : lo + 384].rearrange(
                        "p (c j) -> p c j", c=3)

                otv = otile[hp, :].rearrange("p (t s) -> p s t", s=7)
                nc.vector.scalar_tensor_tensor(
                    otv[:, 4:7, :], pl3(tmpD, 0), 2.0, pl3(uvp, 0),
                    OP.mult, OP.add,
                )
                # quat_vel = quat_mult(q_raw, (0, s)),  s = 0.05*(h2@Wr+br)
                # w: -(u . s) via elementwise mul + inner-plane reduce
                nc.vector.tensor_mul(tmpD[hp, :], qcd[hp, 0:384],
                                     uvp[hp, 640:1024])
                nc.vector.reduce_sum(
                    tmpA[hp, :],
                    tmpD[hp, :].rearrange("p (c j) -> p j c", c=3), axis=AX.X)
                nc.vector.tensor_scalar_mul(otile[hp, 0::7], tmpA[hp, :], -1.0)
                # xyz: qw s_c + (q_{c+1} s_{c+2} - q_{c+2} s_{c+1})
                nc.vector.tensor_mul(tmpD[hp, :], qcd[hp, 128:512],
                                     uvp[hp, 896:1280])
                nc.vector.tensor_mul(tmpE[hp, :], qcd[hp, 256:640],
                                     uvp[hp, 768:1152])
                nc.vector.tensor_sub(tmpD[hp, :], tmpD[hp, :], tmpE[hp, :])
                nc.vector.tensor_mul(tmpE[hp, :], wd[hp, :], uvp[hp, 640:1024])
                nc.vector.tensor_add(otv[:, 1:4, :], pl3(tmpE, 0), pl3(tmpD, 0))
                nc.sync.dma_start(out_d[hp, :], otile[hp, :])

            # ---------- PE warm-up ----------
            # The HAM clock gate needs ~3.4us of sustained PE activity to
            # lift the PE from 1.2 to 2.4 GHz, and re-throttles after ~3.4us
            # idle.  Dummy matmuls bridge the DVE-only head; data-dependent
            # "tracker" matmuls then pace the PE along the DVE chain so the
            # real pipeline starts warm.
            def warm_mm(rhs_ap, n):
                pw = ps2.tile([P, 512], F32, tag="p2", name="warm")
                nc.tensor.matmul(pw[:, 0:n], g128[:, 0:128], rhs_ap,
                                 start=True, stop=True)

            for w in range(4):
                warm_mm(qt[:, 0:512], 512)
            warm_mm(n2[:], 128)                    # after |q|^2 reduce
            warm_mm(inv2d[:, 0:384], 384)
            warm_mm(qcd[:, 0:512], 512)            # after compact planes
            warm_mm(reld[:, 0:384], 384)           # after rel

            # ---------- schedule ----------
            h1s = {}
            lrp_full()
            # trackers: keep the PE busy while the lrp tail + bridge run
            for rng in ((0, 128), (128, 256), (256, 384)):
                pw = ps2.tile([P, 512], F32, tag="p2", name="warm")
                nc.tensor.matmul(pw[0:128, 0 : rng[1] - rng[0]],
                                 w2[0:64, 0:128],
                                 lrp[0:64, rng[0] : rng[1]],
                                 start=True, stop=True)

            l1_half(0, h1s)
            l23_half(0, h1s)
            l1_half(1, h1s)
            out_half(0)          # overlaps half-1 matmuls/gelu
            l23_half(1, h1s)
            out_half(1)

    nc.finalize()
    return nc


def make_in_maps(scalar_features, quat, trans, W1, b1, W2, b2, Wt, bt, Wr, br):
    import ml_dtypes
    f32 = np.float32
    bf16 = ml_dtypes.bfloat16
    sf = np.asarray(scalar_features, f32).reshape(PAIRS, D)
    qf = np.asarray(quat, f32).reshape(PAIRS * R * 4)
    tf = np.asarray(trans, f32).reshape(PAIRS * R * 3)
    W1 = np.asarray(W1, f32)
    W1a = np.ascontiguousarray(W1[:D])
    W1b = np.ascontiguousarray(W1[D:])                     # [3, 256]
    W1b_hi = W1b.astype(bf16)
    W1b_lo = (W1b - W1b_hi.astype(f32)).astype(bf16)
    W1bc = np.zeros((8, 8192), bf16)
    for g in range(PPC):
        for fc in range(2):
            col = 128 * (2 * g + fc)
            W1bc[0:3, col : col + 128] = W1b_hi[:, 128 * fc : 128 * (fc + 1)]
            W1bc[3:6, col : col + 128] = W1b_lo[:, 128 * fc : 128 * (fc + 1)]
    W1aE = np.concatenate([W1a, np.asarray(b1, f32).reshape(1, D)], axis=0)
    W2 = np.ascontiguousarray(np.asarray(W2, f32)).astype(bf16)
    b2t = np.asarray(b2, f32).reshape(128, 1)
    Wtr = np.zeros((128, 32), f32)
    Wtr[:, 0:3] = np.asarray(Wt, f32)
    Wtr[:, 3:6] = 0.05 * np.asarray(Wr, f32)
    Wtr = Wtr.astype(bf16)
    btr = np.zeros((P, 1), f32)
    for m in range(4):
        btr[32 * m : 32 * m + 3, 0] = np.asarray(bt, f32)
        btr[32 * m + 3 : 32 * m + 6, 0] = 0.05 * np.asarray(br, f32)
    G = np.kron(np.eye(32, dtype=f32), np.ones((4, 4), f32))
    ones2 = np.ones((2, 16384), bf16)

    in_maps = []
    for i in range(NCORES):
        sl = slice(PPC * i, PPC * (i + 1))
        sfTe = np.concatenate(
            [np.ascontiguousarray(sf[sl].T), np.ones((1, PPC), f32)], axis=0)
        in_maps.append({
            "quat": np.ascontiguousarray(
                qf[TOK * 4 * i : TOK * 4 * (i + 1)].reshape(P, 512)),
            "trans": np.ascontiguousarray(
                tf[TOK * 3 * i : TOK * 3 * (i + 1)].reshape(P, 384)),
            "sfTe": sfTe, "W1aE": W1aE, "W1bc": W1bc, "ones2": ones2,
            "W2": W2, "b2t": b2t,
            "Wtr": Wtr, "btr": btr, "G": G,
        })
    return in_maps


_NC_CACHE = None


def kernel(**inputs):
    global _NC_CACHE
    if _NC_CACHE is None:
        _NC_CACHE = build_nc()
    in_maps = make_in_maps(**inputs)
    res = run_bass_kernel_spmd(_NC_CACHE, in_maps, list(range(NCORES))).results
    outs = [res[i]["out"].reshape(TOK, 7) for i in range(NCORES)]
    return np.concatenate(outs, axis=0).reshape(B, T, R, 7)


if __name__ == "__main__":
    rng = np.random.default_rng(0)
    ins = {
        "scalar_features": rng.standard_normal((B, T, D), dtype=np.float32),
        "quat": rng.standard_normal((B, T, R, 4), dtype=np.float32),
        "trans": rng.standard_normal((B, T, R, 3), dtype=np.float32),
        "W1": rng.standard_normal((D + 3, D), dtype=np.float32) * 0.06,
        "b1": np.zeros(D, np.float32),
        "W2": rng.standard_normal((D, D // 2), dtype=np.float32) * 0.06,
        "b2": np.zeros(D // 2, np.float32),
        "Wt": rng.standard_normal((D // 2, 3), dtype=np.float32) * 0.09,
        "bt": np.zeros(3, np.float32),
        "Wr": rng.standard_normal((D // 2, 3), dtype=np.float32) * 0.09,
        "br": np.zeros(3, np.float32),
    }
    out = kernel(**ins)
    print("kernel output shape:", out.shape)


# revision 24
# speedup vs baseline: 1.0990x; 1.0039x over previous
"""Trainium2 Bass kernel for nn_EquivariantOutputHead.

Reference computation (B=8, T=32, R=512, D=256):
  x    = broadcast(scalar_features)                      (B,T,R,D)
  rel  = trans - mean_R(trans)
  lrp  = rotate(conj(normalize(quat)), rel)
  h1   = gelu([x, lrp] @ W1 + b1)
  h2   = gelu(h1 @ W2 + b2)
  tv   = rotate(normalize(quat), h2 @ Wt + bt)
  qv   = 0.5 * quat_mult(quat, (0, 0.1*(h2 @ Wr + br)))
  out  = [qv, tv]                                        (B,T,R,7)

Sharding: data-parallel over the 256 (b,t) pairs -> 32 pairs (16384 tokens)
per core.  sf @ W1[:D] + b1 is computed once per (b,t) (tiny matmul) and
folded into layer 1 as two bf16 hi/lo contraction rows (rhs rows = ones);
the layer-1 matmul is K=8 (3 lrp hi + 3 lrp lo + 2 c rows).

Rotations use v' = v +/- 2w(uxv)/|q|^2 + 2ux(uxv)/|q|^2 with raw quat
components - only 1/n2 (DVE reciprocal) is needed, no sqrt table.

The 32 groups are processed in two 16-group halves; all plane (DVE) work
for a half touches only its 64 partitions, so half-1 preprocessing and
half-0 output rotation overlap the matmul/gelu window of the other half.
"""

import os
import sys

for _p in ("/opt/trn_rl_repo",):
    if _p not in sys.path:
        sys.path.insert(0, _p)

import numpy as np

import concourse.bacc as bacc
import concourse.bass as bass
import concourse.mybir as mybir
import concourse.tile as tile
from concourse.bass_utils import run_bass_kernel_spmd

F32 = mybir.dt.float32
BF16 = mybir.dt.bfloat16
AF = mybir.ActivationFunctionType
OP = mybir.AluOpType
AX = mybir.AxisListType

B, T, R, D = 8, 32, 512, 256
NCORES = 8
PAIRS = B * T              # 256 (b,t) pairs
PPC = PAIRS // NCORES      # 32 pairs (groups) per core
TOK = PPC * R              # 16384 tokens per core
P = 128                    # partitions
NBLK = 8                   # uvT blocks (4 groups each)

GELU = AF.Gelu_apprx_tanh


def build_nc():
    nc = bacc.Bacc(None)

    quat_d = nc.declare_dram_parameter("quat", [P, 512], F32, isOutput=False)
    trans_d = nc.declare_dram_parameter("trans", [P, 384], F32, isOutput=False)
    sfTe_d = nc.declare_dram_parameter("sfTe", [257, PPC], F32, isOutput=False)
    w1aE_d = nc.declare_dram_parameter("W1aE", [257, 256], F32, isOutput=False)
    w1bc_d = nc.declare_dram_parameter("W1bc", [8, 8192], BF16, isOutput=False)
    ones_d = nc.declare_dram_parameter("ones2", [2, 16384], BF16, isOutput=False)
    w2_d = nc.declare_dram_parameter("W2", [256, 128], BF16, isOutput=False)
    b2t_d = nc.declare_dram_parameter("b2t", [P, 1], F32, isOutput=False)
    wtr_d = nc.declare_dram_parameter("Wtr", [P, 32], BF16, isOutput=False)
    btr_d = nc.declare_dram_parameter("btr", [P, 1], F32, isOutput=False)
    g_d = nc.declare_dram_parameter("G", [P, P], F32, isOutput=False)
    out_d = nc.declare_dram_parameter("out", [P, 896], F32, isOutput=True)

    with tile.TileContext(nc) as tc:
        with (
            tc.tile_pool(name="main", bufs=1) as main,
            tc.tile_pool(name="act", bufs=4) as actp,
            tc.tile_pool(name="h1p", bufs=18) as h1p,
            tc.tile_pool(name="ps1", bufs=2, space="PSUM") as ps1,
            tc.tile_pool(name="ps2", bufs=3, space="PSUM") as ps2,
            tc.tile_pool(name="ps3", bufs=1, space="PSUM") as ps3,
        ):
            # ---------- persistent SBUF tensors ----------
            qt = main.tile([P, 512], F32, tag="qt")     # raw quat, interleaved
            tt = main.tile([P, 384], F32, tag="tt")     # trans, interleaved
            sfTe = main.tile([P, 3 * PPC], F32, tag="sfTe")
            w1aE = main.tile([P, 768], F32, tag="w1aE")
            lhsT_all = main.tile([8, 8192], BF16, tag="lhsT_all")
            w2 = main.tile([P, 256], BF16, tag="w2")
            b2t = main.tile([P, 1], F32, tag="b2t")
            wtr = main.tile([P, 32], BF16, tag="wtr")
            btr = main.tile([P, 1], F32, tag="btr")
            g128 = main.tile([P, P], F32, tag="g128")

            cRhi = main.tile([PPC, 256], BF16, tag="cRhi")
            cRhf = main.tile([PPC, 256], F32, tag="cRhf")
            cRlo = main.tile([PPC, 256], BF16, tag="cRlo")
            S3 = main.tile([P, 3], F32, tag="S3")
            cent = main.tile([P, 3], F32, tag="cent")
            qc = main.tile([P, 512], F32, tag="qc")      # compact quat planes
            n2 = main.tile([P, P], F32, tag="n2")
            # duplicated-plane layouts: [x y z x y] etc. so cross products
            # batch as single [*,384] DVE ops over 3 contiguous planes
            qcd = main.tile([P, 640], F32, tag="qcd")    # vec planes dup
            wd = main.tile([P, 384], F32, tag="wd")      # w plane x3
            inv2d = main.tile([P, 384], F32, tag="inv2d")  # 1/|q|^2 x3
            reld = main.tile([P, 640], F32, tag="reld")  # rel planes dup
            crd = main.tile([P, 640], F32, tag="crd")    # cross dup
            dd = main.tile([P, 384], F32, tag="dd")
            lrp = main.tile([P, 384], BF16, tag="lrp")
            tmpA = main.tile([P, P], F32, tag="tmpA")
            tmpD = main.tile([P, 384], F32, tag="tmpD")
            tmpE = main.tile([P, 384], F32, tag="tmpE")
            rhsT = main.tile([8, 16384], BF16, tag="rhsT")
            uvT = main.tile([P, 512 * NBLK], F32, tag="uvT")
            # uvp planes: u(0:384) udup(384:640) s(640:1024) sdup(1024:1280)
            uvp = main.tile([P, 1280], F32, tag="uvp")
            otile = main.tile([P, 896], F32, tag="otile")

            # ---------- loads (sync HWDGE; scalar stays pure-ACT) ----------
            # order = need time: qt/tt/g128 gate the DVE chain + warm-ups
            nc.sync.dma_start(qt[:], quat_d[:])
            nc.sync.dma_start(sfTe[:, 0:PPC], sfTe_d[0:128, :])
            nc.sync.dma_start(sfTe[:, PPC : 2 * PPC], sfTe_d[128:256, :])
            nc.sync.dma_start(sfTe[0:1, 2 * PPC : 3 * PPC], sfTe_d[256:257, :])
            nc.sync.dma_start(w1aE[:, 0:256], w1aE_d[0:128, :])
            nc.sync.dma_start(w1aE[:, 256:512], w1aE_d[128:256, :])
            nc.sync.dma_start(w1aE[0:1, 512:768], w1aE_d[256:257, :])
            nc.sync.dma_start(tt[:], trans_d[:])
            nc.sync.dma_start(g128[:], g_d[:])
            nc.sync.dma_start(lhsT_all[:], w1bc_d[:])
            nc.sync.dma_start(rhsT[6:8, :], ones_d[:])
            nc.sync.dma_start(w2[:, 0:128], w2_d[0:128, :])
            nc.sync.dma_start(w2[:, 128:256], w2_d[128:256, :])
            nc.sync.dma_start(b2t[:], b2t_d[:])
            nc.sync.dma_start(wtr[:], wtr_d[:])
            nc.sync.dma_start(btr[:], btr_d[:])

            # preload the gelu table set off the critical path
            nc.scalar.activation(tmpA[0:1, 0:1], qt[0:1, 0:1], GELU)

            # ---------- quat prep (full planes; only needs qt) ----------
            # n2 = |q|^2 via square + inner-axis reduce (2 ops)
            nc.vector.tensor_mul(qc[:], qt[:], qt[:])     # scratch: q^2
            nc.vector.reduce_sum(
                n2[:], qc[:].rearrange("p (j c) -> p j c", c=4), axis=AX.X)
            nc.vector.reciprocal(inv2d[:, 0:128], n2[:])
            nc.vector.tensor_copy(inv2d[:, 128:256], inv2d[:, 0:128])
            nc.vector.tensor_copy(inv2d[:, 256:384], inv2d[:, 0:128])
            for c in range(4):
                nc.vector.tensor_copy(qc[:, P * c : P * (c + 1)], qt[:, c::4])
            nc.vector.tensor_copy(qcd[:, 0:384], qc[:, 128:512])
            nc.vector.tensor_copy(qcd[:, 384:640], qc[:, 128:384])
            nc.vector.tensor_copy(wd[:, 0:128], qc[:, 0:128])
            nc.vector.tensor_copy(wd[:, 128:256], qc[:, 0:128])
            nc.vector.tensor_copy(wd[:, 256:384], qc[:, 0:128])

            # ---------- cR = sf @ W1a + b1, token-major [32, 256] ----------
            # early: the scatter DMA completion gates layer 1
            psc2 = ps2.tile([PPC, 256], F32, tag="p2", name="psc2")
            nc.tensor.matmul(psc2[:], sfTe[:, 0:PPC], w1aE[:, 0:256],
                             start=True, stop=False)
            nc.tensor.matmul(psc2[:], sfTe[:, PPC : 2 * PPC], w1aE[:, 256:512],
                             start=False, stop=False)
            nc.tensor.matmul(psc2[:], sfTe[0:1, 2 * PPC : 3 * PPC],
                             w1aE[0:1, 512:768], start=False, stop=True)
            nc.vector.tensor_copy(cRhi[:], psc2[:])
            nc.vector.tensor_copy(cRhf[:], cRhi[:])
            nc.vector.tensor_sub(cRlo[:], psc2[:], cRhf[:])
            nc.sync.dma_start(lhsT_all[6:7, :], cRhi[:])
            nc.sync.dma_start(lhsT_all[7:8, :], cRlo[:])

            # ---------- centroid prep ----------
            nc.vector.reduce_sum(
                S3[:, 0:3], tt[:].rearrange("p (j c) -> p c j", c=3), axis=AX.X)
            for h in range(2):
                hp = slice(64 * h, 64 * h + 64)
                psch = ps2.tile([P, 3], F32, tag="p2", name="psc")
                nc.tensor.matmul(psch[hp, :], g128[hp, hp], S3[hp, :],
                                 start=True, stop=True,
                                 tile_position=(64 * h, 64 * h))
                nc.vector.tensor_scalar_mul(cent[hp, :], psch[hp, :], 1.0 / 512.0)
            for c in range(3):
                nc.vector.tensor_scalar_sub(
                    reld[:, P * c : P * (c + 1)], tt[:, c::3], cent[:, c : c + 1]
                )
            nc.vector.tensor_copy(reld[:, 384:640], reld[:, 0:256])

            def cross_batched(out_ap, a_dup, b_dup, hp):
                # out = a x b over 3 planes: a_dup/b_dup are [*,640] dup tiles
                nc.vector.tensor_mul(tmpD[hp, :], a_dup[hp, 128:512],
                                     b_dup[hp, 256:640])
                nc.vector.tensor_mul(tmpE[hp, :], a_dup[hp, 256:640],
                                     b_dup[hp, 128:512])
                nc.vector.tensor_sub(out_ap, tmpD[hp, :], tmpE[hp, :])

            def lrp_full():
                # lrp = rotate(conj(q), rel) = rel + (2 ux(uxv) - 2w(uxv))/n2
                hp = slice(0, 128)
                cross_batched(crd[:, 0:384], qcd, reld, hp)
                nc.vector.tensor_copy(crd[:, 384:640], crd[:, 0:256])
                cross_batched(dd[:, 0:384], qcd, crd, hp)
                nc.vector.tensor_mul(tmpD[:], wd[:], crd[:, 0:384])
                nc.vector.tensor_sub(tmpE[:], dd[:], tmpD[:])
                nc.vector.tensor_mul(tmpD[:], tmpE[:], inv2d[:])
                # per-plane finish + bridge: row c col 128p+j = lrp_c[p, j].
                # Each row write lands on one partition (= one SDMA engine),
                # so split rows into halves across both HWDGE queues to
                # pipeline the descriptor-dominated SBUF->SBUF latency.
                for c in range(3):
                    nc.vector.scalar_tensor_tensor(
                        lrp[:, P * c : P * (c + 1)],
                        tmpD[:, P * c : P * (c + 1)], 2.0,
                        reld[:, P * c : P * (c + 1)], OP.mult, OP.add)
                    for r in (c, c + 3):
                        for hh in range(2):
                            eng = nc.sync if (r + hh) % 2 == 0 else nc.scalar
                            eng.dma_start(
                                rhsT[r : r + 1, 8192 * hh : 8192 * (hh + 1)],
                                lrp[64 * hh : 64 * hh + 64,
                                    P * c : P * (c + 1)],
                            )

            def l1_half(h, h1s):
                # layer 1 (K=8), groups g = 16h + 4*sig + bb
                for bb in range(4):
                    for sig in range(4):
                        g = 16 * h + 4 * sig + bb
                        rhs_g = rhsT[:, 512 * g : 512 * (g + 1)]
                        h1 = h1p.tile([P, 1024], BF16, tag="h1", name="h1")
                        p1 = ps1.tile([P, 1024], F32, tag="p1", name="p1")
                        for fc in range(2):
                            nc.tensor.matmul(
                                p1[:, 512 * fc : 512 * (fc + 1)],
                                lhsT_all[:, 128 * (2 * g + fc) : 128 * (2 * g + fc) + 128],
                                rhs_g,
                                start=True, stop=True,
                            )
                        nc.scalar.activation(h1[:], p1[:], GELU)
                        h1s[g] = h1

            def l23_half(h, h1s):
                # layers 2+3 (K=128); psum3 block beta = 4h+bb packs 4 sigs
                for bb in range(4):
                    beta = 4 * h + bb
                    for sig in range(4):
                        g = 16 * h + 4 * sig + bb
                        h1 = h1s[g]
                        p2 = ps2.tile([P, 512], F32, tag="p2", name="p2")
                        for kc in range(2):
                            nc.tensor.matmul(
                                p2[:],
                                w2[:, 128 * kc : 128 * (kc + 1)],
                                h1[:, 512 * kc : 512 * (kc + 1)],
                                start=(kc == 0), stop=(kc == 1),
                            )
                        h2 = actp.tile([P, 512], BF16, tag="h2", name="h2")
                        nc.scalar.activation(h2[:], p2[:], GELU, bias=b2t[:, 0:1])
                        if sig == 0:
                            p3b = ps3.tile([P, 512], F32, tag="p3", name="p3")
                        nc.tensor.matmul(
                            p3b[32 * sig : 32 * sig + 32, :],
                            wtr[:],
                            h2[:],
                            start=True, stop=True,
                            tile_position=(0, 32 * sig),
                        )
                        if sig == 3:
                            nc.vector.tensor_scalar_add(
                                uvT[:, 512 * beta : 512 * (beta + 1)],
                                p3b[:], btr[:, 0:1],
                            )

            def out_half(h):
                hp = slice(64 * h, 64 * h + 64)
                # reverse bridge: uvT[32sig+k, 2048h+512bb+128q+j] ->
                # uvp[64h+16sig+4bb+q, ...]; one DMA per k (4 sig strips).
                # u -> planes 0-2, s -> planes 5-7 (cols 640:1024)
                for k in range(6):
                    dst = P * k if k < 3 else P * k + 256
                    nc.sync.dma_start(
                        uvp[hp, dst : dst + P],
                        uvT[k : k + 97 : 32, 2048 * h : 2048 * (h + 1)],
                    )
                nc.vector.tensor_copy(uvp[hp, 384:640], uvp[hp, 0:256])
                nc.vector.tensor_copy(uvp[hp, 1024:1280], uvp[hp, 640:896])
                # trans_vel = rotate(q, u) = u + (2 ux(uxu) + 2w(uxu))/n2
                cross_batched(crd[hp, 0:384], qcd, uvp, hp)
                nc.vector.tensor_copy(crd[hp, 384:640], crd[hp, 0:256])
                cross_batched(dd[hp, 0:384], qcd, crd, hp)
                nc.vector.tensor_mul(tmpD[hp, :], wd[hp, :], crd[hp, 0:384])
                nc.vector.tensor_add(tmpE[hp, :], dd[hp, :], tmpD[hp, :])
                nc.vector.tensor_mul(tmpD[hp, :], tmpE[hp, :], inv2d[hp, :])
                def pl3(t, lo):
                    return t[hp, lo : lo + 384].rearrange(
                        "p (c j) -> p c j", c=3)

                otv = otile[hp, :].rearrange("p (t s) -> p s t", s=7)
                nc.vector.scalar_tensor_tensor(
                    otv[:, 4:7, :], pl3(tmpD, 0), 2.0, pl3(uvp, 0),
                    OP.mult, OP.add,
                )
                # quat_vel = quat_mult(q_raw, (0, s)),  s = 0.05*(h2@Wr+br)
                # w: -(u . s) via elementwise mul + inner-plane reduce
                nc.vector.tensor_mul(tmpD[hp, :], qcd[hp, 0:384],
                                     uvp[hp, 640:1024])
                nc.vector.reduce_sum(
                    tmpA[hp, :],
                    tmpD[hp, :].rearrange("p (c j) -> p j c", c=3), axis=AX.X)
                nc.vector.tensor_scalar_mul(otile[hp, 0::7], tmpA[hp, :], -1.0)
                # xyz: qw s_c + (q_{c+1} s_{c+2} - q_{c+2} s_{c+1})
                nc.vector.tensor_mul(tmpD[hp, :], qcd[hp, 128:512],
                                     uvp[hp, 896:1280])
                nc.vector.tensor_mul(tmpE[hp, :], qcd[hp, 256:640],
                                     uvp[hp, 768:1152])
                nc.vector.tensor_sub(tmpD[hp, :], tmpD[hp, :], tmpE[hp, :])
                nc.vector.tensor_mul(tmpE[hp, :], wd[hp, :], uvp[hp, 640:1024])
                nc.vector.tensor_add(otv[:, 1:4, :], pl3(tmpE, 0), pl3(tmpD, 0))
                nc.sync.dma_start(out_d[hp, :], otile[hp, :])

            # ---------- PE warm-up ----------
            # The HAM clock gate needs ~3.4us of sustained PE activity to
            # lift the PE from 1.2 to 2.4 GHz, and re-throttles after ~3.4us
            # idle.  Dummy matmuls bridge the DVE-only head; data-dependent
            # "tracker" matmuls then pace the PE along the DVE chain so the
            # real pipeline starts warm.
            def warm_mm(rhs_ap, n):
                pw = ps2.tile([P, 512], F32, tag="p2", name="warm")
                nc.tensor.matmul(pw[:, 0:n], g128[:, 0:128], rhs_ap,
                                 start=True, stop=True)

            for w in range(4):
                warm_mm(qt[:, 0:512], 512)
            warm_mm(n2[:], 128)                    # after |q|^2 reduce
            warm_mm(inv2d[:, 0:384], 384)
            warm_mm(qcd[:, 0:512], 512)            # after compact planes
            warm_mm(reld[:, 0:384], 384)           # after rel

            # ---------- schedule ----------
            h1s = {}
            lrp_full()
            # trackers: keep the PE busy while the lrp tail + bridge run
            for rng in ((0, 128), (128, 256), (256, 384)):
                pw = ps2.tile([P, 512], F32, tag="p2", name="warm")
                nc.tensor.matmul(pw[0:128, 0 : rng[1] - rng[0]],
                                 w2[0:64, 0:128],
                                 lrp[0:64, rng[0] : rng[1]],
                                 start=True, stop=True)

            l1_half(0, h1s)
            l23_half(0, h1s)
            l1_half(1, h1s)
            out_half(0)          # overlaps half-1 matmuls/gelu
            l23_half(1, h1s)
            out_half(1)

    nc.finalize()
    return nc


def make_in_maps(scalar_features, quat, trans, W1, b1, W2, b2, Wt, bt, Wr, br):
    import ml_dtypes
    f32 = np.float32
    bf16 = ml_dtypes.bfloat16
    sf = np.asarray(scalar_features, f32).reshape(PAIRS, D)
    qf = np.asarray(quat, f32).reshape(PAIRS * R * 4)
    tf = np.asarray(trans, f32).reshape(PAIRS * R * 3)
    W1 = np.asarray(W1, f32)
    W1a = np.ascontiguousarray(W1[:D])
    W1b = np.ascontiguousarray(W1[D:])                     # [3, 256]
    W1b_hi = W1b.astype(bf16)
    W1b_lo = (W1b - W1b_hi.astype(f32)).astype(bf16)
    W1bc = np.zeros((8, 8192), bf16)
    for g in range(PPC):
        for fc in range(2):
            col = 128 * (2 * g + fc)
            W1bc[0:3, col : col + 128] = W1b_hi[:, 128 * fc : 128 * (fc + 1)]
            W1bc[3:6, col : col + 128] = W1b_lo[:, 128 * fc : 128 * (fc + 1)]
    W1aE = np.concatenate([W1a, np.asarray(b1, f32).reshape(1, D)], axis=0)
    W2 = np.ascontiguousarray(np.asarray(W2, f32)).astype(bf16)
    b2t = np.asarray(b2, f32).reshape(128, 1)
    Wtr = np.zeros((128, 32), f32)
    Wtr[:, 0:3] = np.asarray(Wt, f32)
    Wtr[:, 3:6] = 0.05 * np.asarray(Wr, f32)
    Wtr = Wtr.astype(bf16)
    btr = np.zeros((P, 1), f32)
    for m in range(4):
        btr[32 * m : 32 * m + 3, 0] = np.asarray(bt, f32)
        btr[32 * m + 3 : 32 * m + 6, 0] = 0.05 * np.asarray(br, f32)
    G = np.kron(np.eye(32, dtype=f32), np.ones((4, 4), f32))
    ones2 = np.ones((2, 16384), bf16)

    in_maps = []
    for i in range(NCORES):
        sl = slice(PPC * i, PPC * (i + 1))
        sfTe = np.concatenate(
            [np.ascontiguousarray(sf[sl].T), np.ones((1, PPC), f32)], axis=0)
        in_maps.append({
            "quat": np.ascontiguousarray(
                qf[TOK * 4 * i : TOK * 4 * (i + 1)].reshape(P, 512)),
            "trans": np.ascontiguousarray(
                tf[TOK * 3 * i : TOK * 3 * (i + 1)].reshape(P, 384)),
            "sfTe": sfTe, "W1aE": W1aE, "W1bc": W1bc, "ones2": ones2,
            "W2": W2, "b2t": b2t,
            "Wtr": Wtr, "btr": btr, "G": G,
        })
    return in_maps


_NC_CACHE = None


def kernel(**inputs):
    global _NC_CACHE
    if _NC_CACHE is None:
        _NC_CACHE = build_nc()
    in_maps = make_in_maps(**inputs)
    res = run_bass_kernel_spmd(_NC_CACHE, in_maps, list(range(NCORES))).results
    outs = [res[i]["out"].reshape(TOK, 7) for i in range(NCORES)]
    return np.concatenate(outs, axis=0).reshape(B, T, R, 7)


if __name__ == "__main__":
    rng = np.random.default_rng(0)
    ins = {
        "scalar_features": rng.standard_normal((B, T, D), dtype=np.float32),
        "quat": rng.standard_normal((B, T, R, 4), dtype=np.float32),
        "trans": rng.standard_normal((B, T, R, 3), dtype=np.float32),
        "W1": rng.standard_normal((D + 3, D), dtype=np.float32) * 0.06,
        "b1": np.zeros(D, np.float32),
        "W2": rng.standard_normal((D, D // 2), dtype=np.float32) * 0.06,
        "b2": np.zeros(D // 2, np.float32),
        "Wt": rng.standard_normal((D // 2, 3), dtype=np.float32) * 0.09,
        "bt": np.zeros(3, np.float32),
        "Wr": rng.standard_normal((D // 2, 3), dtype=np.float32) * 0.09,
        "br": np.zeros(3, np.float32),
    }
    out = kernel(**ins)
    print("kernel output shape:", out.shape)


# revision 25
# speedup vs baseline: 1.0997x; 1.0006x over previous
"""Trainium2 Bass kernel for nn_EquivariantOutputHead.

Reference computation (B=8, T=32, R=512, D=256):
  x    = broadcast(scalar_features)                      (B,T,R,D)
  rel  = trans - mean_R(trans)
  lrp  = rotate(conj(normalize(quat)), rel)
  h1   = gelu([x, lrp] @ W1 + b1)
  h2   = gelu(h1 @ W2 + b2)
  tv   = rotate(normalize(quat), h2 @ Wt + bt)
  qv   = 0.5 * quat_mult(quat, (0, 0.1*(h2 @ Wr + br)))
  out  = [qv, tv]                                        (B,T,R,7)

Sharding: data-parallel over the 256 (b,t) pairs -> 32 pairs (16384 tokens)
per core.  sf @ W1[:D] + b1 is computed once per (b,t) (tiny matmul) and
folded into layer 1 as two bf16 hi/lo contraction rows (rhs rows = ones);
the layer-1 matmul is K=8 (3 lrp hi + 3 lrp lo + 2 c rows).

Rotations use v' = v +/- 2w(uxv)/|q|^2 + 2ux(uxv)/|q|^2 with raw quat
components - only 1/n2 (DVE reciprocal) is needed, no sqrt table.

The 32 groups are processed in two 16-group halves; all plane (DVE) work
for a half touches only its 64 partitions, so half-1 preprocessing and
half-0 output rotation overlap the matmul/gelu window of the other half.
"""

import os
import sys

for _p in ("/opt/trn_rl_repo",):
    if _p not in sys.path:
        sys.path.insert(0, _p)

import numpy as np

import concourse.bacc as bacc
import concourse.bass as bass
import concourse.mybir as mybir
import concourse.tile as tile
from concourse.bass_utils import run_bass_kernel_spmd

F32 = mybir.dt.float32
BF16 = mybir.dt.bfloat16
AF = mybir.ActivationFunctionType
OP = mybir.AluOpType
AX = mybir.AxisListType

B, T, R, D = 8, 32, 512, 256
NCORES = 8
PAIRS = B * T              # 256 (b,t) pairs
PPC = PAIRS // NCORES      # 32 pairs (groups) per core
TOK = PPC * R              # 16384 tokens per core
P = 128                    # partitions
NBLK = 8                   # uvT blocks (4 groups each)

GELU = AF.Gelu_apprx_tanh


def build_nc():
    nc = bacc.Bacc(None)

    quat_d = nc.declare_dram_parameter("quat", [P, 512], F32, isOutput=False)
    trans_d = nc.declare_dram_parameter("trans", [P, 384], F32, isOutput=False)
    sfTe_d = nc.declare_dram_parameter("sfTe", [257, PPC], F32, isOutput=False)
    w1aE_d = nc.declare_dram_parameter("W1aE", [257, 256], F32, isOutput=False)
    w1bc_d = nc.declare_dram_parameter("W1bc", [8, 8192], BF16, isOutput=False)
    ones_d = nc.declare_dram_parameter("ones2", [2, 16384], BF16, isOutput=False)
    w2_d = nc.declare_dram_parameter("W2", [256, 128], BF16, isOutput=False)
    b2t_d = nc.declare_dram_parameter("b2t", [P, 1], F32, isOutput=False)
    wtr_d = nc.declare_dram_parameter("Wtr", [P, 32], BF16, isOutput=False)
    btr_d = nc.declare_dram_parameter("btr", [P, 1], F32, isOutput=False)
    g_d = nc.declare_dram_parameter("G", [P, P], F32, isOutput=False)
    out_d = nc.declare_dram_parameter("out", [P, 896], F32, isOutput=True)

    with tile.TileContext(nc) as tc:
        with (
            tc.tile_pool(name="main", bufs=1) as main,
            tc.tile_pool(name="act", bufs=4) as actp,
            tc.tile_pool(name="h1p", bufs=18) as h1p,
            tc.tile_pool(name="ps1", bufs=2, space="PSUM") as ps1,
            tc.tile_pool(name="ps2", bufs=2, space="PSUM") as ps2,
            tc.tile_pool(name="ps3", bufs=2, space="PSUM") as ps3,
        ):
            # ---------- persistent SBUF tensors ----------
            qt = main.tile([P, 512], F32, tag="qt")     # raw quat, interleaved
            tt = main.tile([P, 384], F32, tag="tt")     # trans, interleaved
            sfTe = main.tile([P, 3 * PPC], F32, tag="sfTe")
            w1aE = main.tile([P, 768], F32, tag="w1aE")
            lhsT_all = main.tile([8, 8192], BF16, tag="lhsT_all")
            w2 = main.tile([P, 256], BF16, tag="w2")
            b2t = main.tile([P, 1], F32, tag="b2t")
            wtr = main.tile([P, 32], BF16, tag="wtr")
            btr = main.tile([P, 1], F32, tag="btr")
            g128 = main.tile([P, P], F32, tag="g128")

            cRhi = main.tile([PPC, 256], BF16, tag="cRhi")
            cRhf = main.tile([PPC, 256], F32, tag="cRhf")
            cRlo = main.tile([PPC, 256], BF16, tag="cRlo")
            S3 = main.tile([P, 3], F32, tag="S3")
            cent = main.tile([P, 3], F32, tag="cent")
            qc = main.tile([P, 512], F32, tag="qc")      # compact quat planes
            n2 = main.tile([P, P], F32, tag="n2")
            # duplicated-plane layouts: [x y z x y] etc. so cross products
            # batch as single [*,384] DVE ops over 3 contiguous planes
            qcd = main.tile([P, 640], F32, tag="qcd")    # vec planes dup
            wd = main.tile([P, 384], F32, tag="wd")      # w plane x3
            inv2d = main.tile([P, 384], F32, tag="inv2d")  # 1/|q|^2 x3
            reld = main.tile([P, 640], F32, tag="reld")  # rel planes dup
            crd = main.tile([P, 640], F32, tag="crd")    # cross dup
            dd = main.tile([P, 384], F32, tag="dd")
            lrp = main.tile([P, 384], BF16, tag="lrp")
            tmpA = main.tile([P, P], F32, tag="tmpA")
            tmpD = main.tile([P, 384], F32, tag="tmpD")
            tmpE = main.tile([P, 384], F32, tag="tmpE")
            rhsT = main.tile([8, 16384], BF16, tag="rhsT")
            uvT = main.tile([P, 512 * NBLK], F32, tag="uvT")
            # uvp planes: u(0:384) udup(384:640) s(640:1024) sdup(1024:1280)
            uvp = main.tile([P, 1280], F32, tag="uvp")
            otile = main.tile([P, 896], F32, tag="otile")

            # ---------- loads (sync HWDGE; scalar stays pure-ACT) ----------
            # order = need time: qt/tt/g128 gate the DVE chain + warm-ups
            nc.sync.dma_start(qt[:], quat_d[:])
            nc.sync.dma_start(sfTe[:, 0:PPC], sfTe_d[0:128, :])
            nc.sync.dma_start(sfTe[:, PPC : 2 * PPC], sfTe_d[128:256, :])
            nc.sync.dma_start(sfTe[0:1, 2 * PPC : 3 * PPC], sfTe_d[256:257, :])
            nc.sync.dma_start(w1aE[:, 0:256], w1aE_d[0:128, :])
            nc.sync.dma_start(w1aE[:, 256:512], w1aE_d[128:256, :])
            nc.sync.dma_start(w1aE[0:1, 512:768], w1aE_d[256:257, :])
            nc.sync.dma_start(tt[:], trans_d[:])
            nc.sync.dma_start(g128[:], g_d[:])
            nc.sync.dma_start(lhsT_all[:], w1bc_d[:])
            nc.sync.dma_start(rhsT[6:8, :], ones_d[:])
            nc.sync.dma_start(w2[:, 0:128], w2_d[0:128, :])
            nc.sync.dma_start(w2[:, 128:256], w2_d[128:256, :])
            nc.sync.dma_start(b2t[:], b2t_d[:])
            nc.sync.dma_start(wtr[:], wtr_d[:])
            nc.sync.dma_start(btr[:], btr_d[:])

            # preload the gelu table set off the critical path
            nc.scalar.activation(tmpA[0:1, 0:1], qt[0:1, 0:1], GELU)

            # ---------- quat prep (full planes; only needs qt) ----------
            # n2 = |q|^2 via square + inner-axis reduce (2 ops)
            nc.vector.tensor_mul(qc[:], qt[:], qt[:])     # scratch: q^2
            nc.vector.reduce_sum(
                n2[:], qc[:].rearrange("p (j c) -> p j c", c=4), axis=AX.X)
            nc.vector.reciprocal(inv2d[:, 0:128], n2[:])
            nc.vector.tensor_copy(inv2d[:, 128:256], inv2d[:, 0:128])
            nc.vector.tensor_copy(inv2d[:, 256:384], inv2d[:, 0:128])
            for c in range(4):
                nc.vector.tensor_copy(qc[:, P * c : P * (c + 1)], qt[:, c::4])
            nc.vector.tensor_copy(qcd[:, 0:384], qc[:, 128:512])
            nc.vector.tensor_copy(qcd[:, 384:640], qc[:, 128:384])
            nc.vector.tensor_copy(wd[:, 0:128], qc[:, 0:128])
            nc.vector.tensor_copy(wd[:, 128:256], qc[:, 0:128])
            nc.vector.tensor_copy(wd[:, 256:384], qc[:, 0:128])

            # ---------- cR = sf @ W1a + b1, token-major [32, 256] ----------
            # early: the scatter DMA completion gates layer 1
            psc2 = ps2.tile([PPC, 256], F32, tag="p2", name="psc2")
            nc.tensor.matmul(psc2[:], sfTe[:, 0:PPC], w1aE[:, 0:256],
                             start=True, stop=False)
            nc.tensor.matmul(psc2[:], sfTe[:, PPC : 2 * PPC], w1aE[:, 256:512],
                             start=False, stop=False)
            nc.tensor.matmul(psc2[:], sfTe[0:1, 2 * PPC : 3 * PPC],
                             w1aE[0:1, 512:768], start=False, stop=True)
            nc.vector.tensor_copy(cRhi[:], psc2[:])
            nc.vector.tensor_copy(cRhf[:], cRhi[:])
            nc.vector.tensor_sub(cRlo[:], psc2[:], cRhf[:])
            nc.sync.dma_start(lhsT_all[6:7, :], cRhi[:])
            nc.sync.dma_start(lhsT_all[7:8, :], cRlo[:])

            # ---------- centroid prep ----------
            nc.vector.reduce_sum(
                S3[:, 0:3], tt[:].rearrange("p (j c) -> p c j", c=3), axis=AX.X)
            for h in range(2):
                hp = slice(64 * h, 64 * h + 64)
                psch = ps2.tile([P, 3], F32, tag="p2", name="psc")
                nc.tensor.matmul(psch[hp, :], g128[hp, hp], S3[hp, :],
                                 start=True, stop=True,
                                 tile_position=(64 * h, 64 * h))
                nc.vector.tensor_scalar_mul(cent[hp, :], psch[hp, :], 1.0 / 512.0)
            for c in range(3):
                nc.vector.tensor_scalar_sub(
                    reld[:, P * c : P * (c + 1)], tt[:, c::3], cent[:, c : c + 1]
                )
            nc.vector.tensor_copy(reld[:, 384:640], reld[:, 0:256])

            def cross_batched(out_ap, a_dup, b_dup, hp):
                # out = a x b over 3 planes: a_dup/b_dup are [*,640] dup tiles
                nc.vector.tensor_mul(tmpD[hp, :], a_dup[hp, 128:512],
                                     b_dup[hp, 256:640])
                nc.vector.tensor_mul(tmpE[hp, :], a_dup[hp, 256:640],
                                     b_dup[hp, 128:512])
                nc.vector.tensor_sub(out_ap, tmpD[hp, :], tmpE[hp, :])

            def lrp_full():
                # lrp = rotate(conj(q), rel) = rel + (2 ux(uxv) - 2w(uxv))/n2
                hp = slice(0, 128)
                cross_batched(crd[:, 0:384], qcd, reld, hp)
                nc.vector.tensor_copy(crd[:, 384:640], crd[:, 0:256])
                cross_batched(dd[:, 0:384], qcd, crd, hp)
                nc.vector.tensor_mul(tmpD[:], wd[:], crd[:, 0:384])
                nc.vector.tensor_sub(tmpE[:], dd[:], tmpD[:])
                nc.vector.tensor_mul(tmpD[:], tmpE[:], inv2d[:])
                # per-plane finish + bridge: row c col 128p+j = lrp_c[p, j].
                # Each row write lands on one partition (= one SDMA engine),
                # so split rows into halves across both HWDGE queues to
                # pipeline the descriptor-dominated SBUF->SBUF latency.
                for c in range(3):
                    nc.vector.scalar_tensor_tensor(
                        lrp[:, P * c : P * (c + 1)],
                        tmpD[:, P * c : P * (c + 1)], 2.0,
                        reld[:, P * c : P * (c + 1)], OP.mult, OP.add)
                    for r in (c, c + 3):
                        for hh in range(2):
                            eng = nc.sync if (r + hh) % 2 == 0 else nc.scalar
                            eng.dma_start(
                                rhsT[r : r + 1, 8192 * hh : 8192 * (hh + 1)],
                                lrp[64 * hh : 64 * hh + 64,
                                    P * c : P * (c + 1)],
                            )

            def l1_half(h, h1s):
                # layer 1 (K=8), groups g = 16h + 4*sig + bb
                for bb in range(4):
                    for sig in range(4):
                        g = 16 * h + 4 * sig + bb
                        rhs_g = rhsT[:, 512 * g : 512 * (g + 1)]
                        h1 = h1p.tile([P, 1024], BF16, tag="h1", name="h1")
                        p1 = ps1.tile([P, 1024], F32, tag="p1", name="p1")
                        for fc in range(2):
                            nc.tensor.matmul(
                                p1[:, 512 * fc : 512 * (fc + 1)],
                                lhsT_all[:, 128 * (2 * g + fc) : 128 * (2 * g + fc) + 128],
                                rhs_g,
                                start=True, stop=True,
                            )
                        nc.scalar.activation(h1[:], p1[:], GELU)
                        h1s[g] = h1

            def l23_half(h, h1s):
                # layers 2+3 (K=128); psum3 block beta = 4h+bb packs 4 sigs
                for bb in range(4):
                    beta = 4 * h + bb
                    for sig in range(4):
                        g = 16 * h + 4 * sig + bb
                        h1 = h1s[g]
                        p2 = ps2.tile([P, 512], F32, tag="p2", name="p2")
                        for kc in range(2):
                            nc.tensor.matmul(
                                p2[:],
                                w2[:, 128 * kc : 128 * (kc + 1)],
                                h1[:, 512 * kc : 512 * (kc + 1)],
                                start=(kc == 0), stop=(kc == 1),
                            )
                        h2 = actp.tile([P, 512], BF16, tag="h2", name="h2")
                        nc.scalar.activation(h2[:], p2[:], GELU, bias=b2t[:, 0:1])
                        if sig == 0:
                            p3b = ps3.tile([P, 512], F32, tag="p3", name="p3")
                        nc.tensor.matmul(
                            p3b[32 * sig : 32 * sig + 32, :],
                            wtr[:],
                            h2[:],
                            start=True, stop=True,
                            tile_position=(0, 32 * sig),
                        )
                        if sig == 3:
                            nc.vector.tensor_scalar_add(
                                uvT[:, 512 * beta : 512 * (beta + 1)],
                                p3b[:], btr[:, 0:1],
                            )

            def out_half(h):
                hp = slice(64 * h, 64 * h + 64)
                # reverse bridge: uvT[32sig+k, 2048h+512bb+128q+j] ->
                # uvp[64h+16sig+4bb+q, ...]; one DMA per k (4 sig strips).
                # u -> planes 0-2, s -> planes 5-7 (cols 640:1024)
                for k in range(6):
                    dst = P * k if k < 3 else P * k + 256
                    nc.sync.dma_start(
                        uvp[hp, dst : dst + P],
                        uvT[k : k + 97 : 32, 2048 * h : 2048 * (h + 1)],
                    )
                nc.vector.tensor_copy(uvp[hp, 384:640], uvp[hp, 0:256])
                nc.vector.tensor_copy(uvp[hp, 1024:1280], uvp[hp, 640:896])
                # trans_vel = rotate(q, u) = u + (2 ux(uxu) + 2w(uxu))/n2
                cross_batched(crd[hp, 0:384], qcd, uvp, hp)
                nc.vector.tensor_copy(crd[hp, 384:640], crd[hp, 0:256])
                cross_batched(dd[hp, 0:384], qcd, crd, hp)
                nc.vector.tensor_mul(tmpD[hp, :], wd[hp, :], crd[hp, 0:384])
                nc.vector.tensor_add(tmpE[hp, :], dd[hp, :], tmpD[hp, :])
                nc.vector.tensor_mul(tmpD[hp, :], tmpE[hp, :], inv2d[hp, :])
                def pl3(t, lo):
                    return t[hp, lo : lo + 384].rearrange(
                        "p (c j) -> p c j", c=3)

                otv = otile[hp, :].rearrange("p (t s) -> p s t", s=7)
                nc.vector.scalar_tensor_tensor(
                    otv[:, 4:7, :], pl3(tmpD, 0), 2.0, pl3(uvp, 0),
                    OP.mult, OP.add,
                )
                # quat_vel = quat_mult(q_raw, (0, s)),  s = 0.05*(h2@Wr+br)
                # w: -(u . s) via elementwise mul + inner-plane reduce
                nc.vector.tensor_mul(tmpD[hp, :], qcd[hp, 0:384],
                                     uvp[hp, 640:1024])
                nc.vector.reduce_sum(
                    tmpA[hp, :],
                    tmpD[hp, :].rearrange("p (c j) -> p j c", c=3), axis=AX.X)
                nc.vector.tensor_scalar_mul(otile[hp, 0::7], tmpA[hp, :], -1.0)
                # xyz: qw s_c + (q_{c+1} s_{c+2} - q_{c+2} s_{c+1})
                nc.vector.tensor_mul(tmpD[hp, :], qcd[hp, 128:512],
                                     uvp[hp, 896:1280])
                nc.vector.tensor_mul(tmpE[hp, :], qcd[hp, 256:640],
                                     uvp[hp, 768:1152])
                nc.vector.tensor_sub(tmpD[hp, :], tmpD[hp, :], tmpE[hp, :])
                nc.vector.tensor_mul(tmpE[hp, :], wd[hp, :], uvp[hp, 640:1024])
                nc.vector.tensor_add(otv[:, 1:4, :], pl3(tmpE, 0), pl3(tmpD, 0))
                nc.sync.dma_start(out_d[hp, :], otile[hp, :])

            # ---------- PE warm-up ----------
            # The HAM clock gate needs ~3.4us of sustained PE activity to
            # lift the PE from 1.2 to 2.4 GHz, and re-throttles after ~3.4us
            # idle.  Dummy matmuls bridge the DVE-only head; data-dependent
            # "tracker" matmuls then pace the PE along the DVE chain so the
            # real pipeline starts warm.
            def warm_mm(rhs_ap, n):
                pw = ps2.tile([P, 512], F32, tag="p2", name="warm")
                nc.tensor.matmul(pw[:, 0:n], g128[:, 0:128], rhs_ap,
                                 start=True, stop=True)

            for w in range(4):
                warm_mm(qt[:, 0:512], 512)
            warm_mm(n2[:], 128)                    # after |q|^2 reduce
            warm_mm(inv2d[:, 0:384], 384)
            warm_mm(qcd[:, 0:512], 512)            # after compact planes
            warm_mm(reld[:, 0:384], 384)           # after rel

            # ---------- schedule ----------
            h1s = {}
            lrp_full()
            # trackers: keep the PE busy while the lrp tail + bridge run
            for rng in ((0, 128), (128, 256), (256, 384)):
                pw = ps2.tile([P, 512], F32, tag="p2", name="warm")
                nc.tensor.matmul(pw[0:128, 0 : rng[1] - rng[0]],
                                 w2[0:64, 0:128],
                                 lrp[0:64, rng[0] : rng[1]],
                                 start=True, stop=True)

            l1_half(0, h1s)
            l23_half(0, h1s)
            l1_half(1, h1s)
            out_half(0)          # overlaps half-1 matmuls/gelu
            l23_half(1, h1s)
            out_half(1)

    nc.finalize()
    return nc


def make_in_maps(scalar_features, quat, trans, W1, b1, W2, b2, Wt, bt, Wr, br):
    import ml_dtypes
    f32 = np.float32
    bf16 = ml_dtypes.bfloat16
    sf = np.asarray(scalar_features, f32).reshape(PAIRS, D)
    qf = np.asarray(quat, f32).reshape(PAIRS * R * 4)
    tf = np.asarray(trans, f32).reshape(PAIRS * R * 3)
    W1 = np.asarray(W1, f32)
    W1a = np.ascontiguousarray(W1[:D])
    W1b = np.ascontiguousarray(W1[D:])                     # [3, 256]
    W1b_hi = W1b.astype(bf16)
    W1b_lo = (W1b - W1b_hi.astype(f32)).astype(bf16)
    W1bc = np.zeros((8, 8192), bf16)
    for g in range(PPC):
        for fc in range(2):
            col = 128 * (2 * g + fc)
            W1bc[0:3, col : col + 128] = W1b_hi[:, 128 * fc : 128 * (fc + 1)]
            W1bc[3:6, col : col + 128] = W1b_lo[:, 128 * fc : 128 * (fc + 1)]
    W1aE = np.concatenate([W1a, np.asarray(b1, f32).reshape(1, D)], axis=0)
    W2 = np.ascontiguousarray(np.asarray(W2, f32)).astype(bf16)
    b2t = np.asarray(b2, f32).reshape(128, 1)
    Wtr = np.zeros((128, 32), f32)
    Wtr[:, 0:3] = np.asarray(Wt, f32)
    Wtr[:, 3:6] = 0.05 * np.asarray(Wr, f32)
    Wtr = Wtr.astype(bf16)
    btr = np.zeros((P, 1), f32)
    for m in range(4):
        btr[32 * m : 32 * m + 3, 0] = np.asarray(bt, f32)
        btr[32 * m + 3 : 32 * m + 6, 0] = 0.05 * np.asarray(br, f32)
    G = np.kron(np.eye(32, dtype=f32), np.ones((4, 4), f32))
    ones2 = np.ones((2, 16384), bf16)

    in_maps = []
    for i in range(NCORES):
        sl = slice(PPC * i, PPC * (i + 1))
        sfTe = np.concatenate(
            [np.ascontiguousarray(sf[sl].T), np.ones((1, PPC), f32)], axis=0)
        in_maps.append({
            "quat": np.ascontiguousarray(
                qf[TOK * 4 * i : TOK * 4 * (i + 1)].reshape(P, 512)),
            "trans": np.ascontiguousarray(
                tf[TOK * 3 * i : TOK * 3 * (i + 1)].reshape(P, 384)),
            "sfTe": sfTe, "W1aE": W1aE, "W1bc": W1bc, "ones2": ones2,
            "W2": W2, "b2t": b2t,
            "Wtr": Wtr, "btr": btr, "G": G,
        })
    return in_maps


_NC_CACHE = None


def kernel(**inputs):
    global _NC_CACHE
    if _NC_CACHE is None:
        _NC_CACHE = build_nc()
    in_maps = make_in_maps(**inputs)
    res = run_bass_kernel_spmd(_NC_CACHE, in_maps, list(range(NCORES))).results
    outs = [res[i]["out"].reshape(TOK, 7) for i in range(NCORES)]
    return np.concatenate(outs, axis=0).reshape(B, T, R, 7)


if __name__ == "__main__":
    rng = np.random.default_rng(0)
    ins = {
        "scalar_features": rng.standard_normal((B, T, D), dtype=np.float32),
        "quat": rng.standard_normal((B, T, R, 4), dtype=np.float32),
        "trans": rng.standard_normal((B, T, R, 3), dtype=np.float32),
        "W1": rng.standard_normal((D + 3, D), dtype=np.float32) * 0.06,
        "b1": np.zeros(D, np.float32),
        "W2": rng.standard_normal((D, D // 2), dtype=np.float32) * 0.06,
        "b2": np.zeros(D // 2, np.float32),
        "Wt": rng.standard_normal((D // 2, 3), dtype=np.float32) * 0.09,
        "bt": np.zeros(3, np.float32),
        "Wr": rng.standard_normal((D // 2, 3), dtype=np.float32) * 0.09,
        "br": np.zeros(3, np.float32),
    }
    out = kernel(**ins)
    print("kernel output shape:", out.shape)
